# revision 1
# baseline (speedup 1.0000x reference)
"""Trainium2 Bass kernel for DiffVorticeSketchRender.

Sharding: 8 cores = 4 batches x 2 H-halves (64 rows each + 3-4 row halos).
Device layout: [D=128 partitions, H slices, W free] everywhere.
- curl + fdiffs: PSUM-accumulated matmuls with +/-I and D-difference band
  matrices (H/W shifts via shifted rhs access patterns, W edge handled by a
  host-extrapolated 129th column, D edge inside the band matrix).
- 3D gaussian smooth (separable): 7 accumulated matmuls fuse the D-conv
  (band matrix) with the H-conv (shifted slice windows), then 7 accumulated
  identity matmuls with shifted W windows for the W-conv.
- depth flip + cumsum: one suffix-sum triangular matmul.
- transmittance/integration: exp on ScalarE, band-matrix matmul for the
  trapezoid coefficients, ones/e127 reduction matmuls. All fp32r, N>=256.
"""

import numpy as np

import concourse.bacc as bacc
import concourse.bass as bass
import concourse.mybir as mybir
import concourse.tile as tile
from concourse.bass_utils import run_bass_kernel_spmd

F32 = mybir.dt.float32
F32R = mybir.dt.float32r
AL = mybir.AluOpType
AF = mybir.ActivationFunctionType

KHS, SIGMA, C = 3, 1.6, 20.0


def _gauss1d():
    size = 2 * KHS + 1
    g = np.arange(size, dtype=np.float64) - (size - 1) / 2.0
    g = np.exp(-((g / SIGMA) ** 2) / 2.0) / (SIGMA * np.sqrt(2.0 * np.pi))
    return (g / g.sum()).astype(np.float32)


GK = _gauss1d()


def _const_mats():
    mdz = np.zeros((128, 128), np.float32)
    for d in range(127):
        mdz[d, d] = -1.0
        mdz[d, d + 1] = 1.0
    mdz[127, 126] = -1.0
    mdz[127, 127] = 1.0

    bd = np.zeros((128, 128), np.float32)
    for dp in range(128):
        for k in range(7):
            d = dp + k - 3
            if 0 <= d < 128:
                bd[dp, d] = GK[k]

    mc = np.zeros((128, 128), np.float32)
    mc[0, 0], mc[0, 1] = -0.5, 0.5
    for k in range(1, 127):
        mc[k, k - 1], mc[k, k + 1] = -0.5, 0.5
    mc[127, 126], mc[127, 127] = -0.5, -0.5

    eye = np.eye(128, dtype=np.float32)
    kbd = np.stack([(GK[k] * bd).T for k in range(7)], axis=1)  # [128,7,128] lhsT, D+H pass
    ki = np.stack([GK[k] * eye for k in range(7)], axis=1)      # [128,7,128] lhsT, W pass
    suf = (np.arange(128)[:, None] >= np.arange(128)[None, :]).astype(np.float32)
    red = np.zeros((128, 2), np.float32)
    red[:, 0] = 1.0
    red[127, 1] = 1.0
    return {
        "KBD": kbd, "KI": ki, "CIP": eye, "CIN": -eye,
        "MDZT": mdz.T.copy(), "MDZTN": (-mdz.T).copy(),
        "SUF": suf, "MCT": mc.T.copy(), "RED": red,
    }


def _curl_groups():
    gs = []
    s0 = 0
    while s0 < 70:
        cnt = min(4, 70 - s0)
        gs.append((s0, cnt))
        s0 += cnt
    return gs


def build_program():
    nc = bacc.Bacc("TRN2", target_bir_lowering=False, debug=False)

    d_in = nc.dram_tensor("d_in", [128, 70, 128], F32R, kind="ExternalInput")
    v_in = nc.dram_tensor("v_in", [3, 128, 71, 129], F32R, kind="ExternalInput")
    m0_in = nc.dram_tensor("m0_in", [128, 3, 128], F32, kind="ExternalInput")
    m1_in = nc.dram_tensor("m1_in", [128, 3, 128], F32, kind="ExternalInput")
    cm = _const_mats()
    c_in = {}
    for name, arr in cm.items():
        c_in[name] = nc.dram_tensor(f"c_{name}", list(arr.shape), F32R,
                                    kind="ExternalInput")
    zpad_in = nc.dram_tensor("zpad", [128, 64, 6], F32R, kind="ExternalInput")
    out_t = nc.dram_tensor("out", [1, 8192], F32, kind="ExternalOutput")

    with tile.TileContext(nc) as tc:
        with tc.tile_pool(name="const", bufs=1) as cpool, \
             tc.tile_pool(name="vols", bufs=1) as vol:
            ct = {}
            for name, arr in cm.items():
                t = cpool.tile(list(arr.shape), F32R, tag=f"c_{name}")
                nc.sync.dma_start(t[:], c_in[name][:])
                ct[name] = t
            m0t = cpool.tile([128, 3, 128], F32, tag="m0")
            m1t = cpool.tile([128, 3, 128], F32, tag="m1")
            nc.sync.dma_start(m0t[:], m0_in[:])
            nc.sync.dma_start(m1t[:], m1_in[:])

            vn = vol.tile([128, 70, 128], F32R, tag="vn")

            # ---- stage 1: curl + |curl|^2 (scoped so v frees after) ----
            with tc.tile_pool(name="vdata", bufs=1) as vp, \
                 tc.tile_pool(name="sq", bufs=4) as sqp, \
                 tc.tile_pool(name="cpsum", bufs=2,
                              space=bass.MemorySpace.PSUM) as cps:
                du = vp.tile([128, 71, 129], F32R, tag="du")
                dv = vp.tile([128, 71, 129], F32R, tag="dv")
                dw = vp.tile([128, 71, 129], F32R, tag="dw")
                # chunk channel loads so early curl groups overlap the DMA
                for a, b in ((0, 6), (6, 13), (13, 25), (25, 37),
                             (37, 49), (49, 61), (61, 71)):
                    nc.sync.dma_start(du[:, a:b, :], v_in[0, :, a:b, :])
                    nc.sync.dma_start(dv[:, a:b, :], v_in[1, :, a:b, :])
                    nc.sync.dma_start(dw[:, a:b, :], v_in[2, :, a:b, :])

                for (s0, cnt) in _curl_groups():
                    n = cnt * 128
                    pcu = cps.tile([128, cnt, 128], F32, tag="pcu")
                    pcv = cps.tile([128, cnt, 128], F32, tag="pcv")
                    pcw = cps.tile([128, cnt, 128], F32, tag="pcw")
                    nc.tensor.matmul(pcu[:], ct["CIP"][:],
                                     dw[:, s0 + 1:s0 + 1 + cnt, 0:128],
                                     start=True, stop=False)
                    nc.tensor.matmul(pcu[:], ct["CIN"][:],
                                     dw[:, s0:s0 + cnt, 0:128],
                                     start=False, stop=False)
                    nc.tensor.matmul(pcu[:], ct["MDZTN"][:],
                                     dv[:, s0:s0 + cnt, 0:128], start=False, stop=True)

                    nc.tensor.matmul(pcv[:], ct["MDZT"][:],
                                     du[:, s0:s0 + cnt, 0:128], start=True, stop=False)
                    nc.tensor.matmul(pcv[:], ct["CIN"][:],
                                     dw[:, s0:s0 + cnt, 1:129],
                                     start=False, stop=False)
                    nc.tensor.matmul(pcv[:], ct["CIP"][:],
                                     dw[:, s0:s0 + cnt, 0:128], start=False, stop=True)

                    nc.tensor.matmul(pcw[:], ct["CIP"][:],
                                     dv[:, s0:s0 + cnt, 1:129], start=True, stop=False)
                    nc.tensor.matmul(pcw[:], ct["CIN"][:],
                                     dv[:, s0:s0 + cnt, 0:128],
                                     start=False, stop=False)
                    nc.tensor.matmul(pcw[:], ct["CIN"][:],
                                     du[:, s0 + 1:s0 + 1 + cnt, 0:128],
                                     start=False, stop=False)
                    nc.tensor.matmul(pcw[:], ct["CIP"][:],
                                     du[:, s0:s0 + cnt, 0:128], start=False, stop=True)

                    squ = sqp.tile([128, cnt, 128], F32, tag="squ")
                    sqv = sqp.tile([128, cnt, 128], F32, tag="sqv")
                    sqw = sqp.tile([128, cnt, 128], F32, tag="sqw")
                    nc.scalar.activation(squ[:], pcu[:], AF.Square)
                    nc.scalar.activation(sqv[:], pcv[:], AF.Square)
                    nc.scalar.activation(sqw[:], pcw[:], AF.Square)
                    tsum = sqp.tile([128, cnt, 128], F32, tag="tsum")
                    nc.vector.tensor_add(tsum[:], squ[:], sqv[:])
                    nc.vector.tensor_add(vn[:, s0:s0 + cnt, :],
                                         tsum[:], sqw[:])

            # mask out-of-range boundary slices, then sqrt in place
            nc.vector.tensor_mul(vn[:, 0:3, :], vn[:, 0:3, :], m0t[:])
            nc.vector.tensor_mul(vn[:, 67:70, :], vn[:, 67:70, :], m1t[:])
            for a, b in ((0, 20), (20, 37), (37, 54), (54, 70)):
                nc.scalar.activation(vn[:, a:b, :], vn[:, a:b, :], AF.Sqrt)

            # ---- stage 2/3: the two 3D smooths ----
            smp_cm = tc.tile_pool(name="smoothp", bufs=1)
            smp = smp_cm.__enter__()
            s1 = smp.tile([128, 64, 134], F32R, tag="s1")
            s1d = smp.tile([128, 64, 134], F32R, tag="s1d")
            for t in (s1, s1d):
                nc.sync.dma_start(t[:, :, 0:3], zpad_in[:, :, 0:3])
                nc.sync.dma_start(t[:, :, 131:134], zpad_in[:, :, 3:6])
            vns = smp.tile([128, 64, 128], F32R, tag="vns")
            dd = smp.tile([128, 70, 128], F32R, tag="dd")
            nc.sync.dma_start(dd[:], d_in[:])
            ds = smp.tile([128, 64, 128], F32R, tag="dd")

            def smooth(src, dst, s1):
                with tc.tile_pool(name="spsum", bufs=3,
                                  space=bass.MemorySpace.PSUM) as sps:
                    for go in range(16):
                        g4 = go * 4
                        p1 = sps.tile([128, 4, 128], F32, tag="p1")
                        for k in range(7):
                            nc.tensor.matmul(p1[:], ct["KBD"][:, k, :],
                                             src[:, g4 + k:g4 + k + 4, :],
                                             start=(k == 0), stop=(k == 6))
                        if go % 2 == 0:
                            nc.scalar.copy(s1[:, g4:g4 + 4, 3:131], p1[:])
                        else:
                            nc.vector.tensor_copy(s1[:, g4:g4 + 4, 3:131],
                                                  p1[:])
                    for go in range(16):
                        g4 = go * 4
                        p2 = sps.tile([128, 4, 128], F32, tag="p2")
                        for k in range(7):
                            nc.tensor.matmul(p2[:], ct["KI"][:, k, :],
                                             s1[:, g4:g4 + 4, k:k + 128],
                                             start=(k == 0), stop=(k == 6))
                        if go % 2 == 0:
                            nc.vector.tensor_copy(dst[:, g4:g4 + 4, :], p2[:])
                        else:
                            nc.scalar.copy(dst[:, g4:g4 + 4, :], p2[:])

            smooth(vn, vns, s1)
            smooth(dd, ds, s1d)

            # ---- stage 4: transmittance + trapezoid integration ----
            ivsb = smp.tile([1, 8192], F32, tag="s1")
            with tc.tile_pool(name="post", bufs=3) as pp, \
                 tc.tile_pool(name="ppsum", bufs=2,
                              space=bass.MemorySpace.PSUM) as pps:
                for cc in range(16):
                    g4 = cc * 4
                    ps = pps.tile([128, 4, 128], F32, tag="ps")
                    nc.tensor.matmul(ps[:], ct["SUF"][:], ds[:, g4:g4 + 4, :],
                                     start=True, stop=True)
                    ec = pp.tile([128, 4, 128], F32R, tag="ec")
                    bc = pp.tile([128, 4, 128], F32R, tag="bc")
                    nc.scalar.activation(ec[:], ps[:], AF.Exp, scale=-C)
                    nc.scalar.activation(bc[:], ps[:], AF.Copy, bias=1.0,
                                         scale=C)
                    nc.vector.tensor_mul(bc[:], bc[:], ec[:])
                    pc2 = pps.tile([128, 4, 128], F32, tag="pc2")
                    nc.tensor.matmul(pc2[:], ct["MCT"][:], bc[:],
                                     start=True, stop=True)
                    pchunk = pp.tile([128, 4, 128], F32R, tag="pchunk")
                    nc.vector.tensor_mul(pchunk[:], pc2[:],
                                         vns[:, g4:g4 + 4, :])
                    piv = pps.tile([1, 512], F32, tag="piv")
                    nc.tensor.matmul(piv[:], ct["RED"][:, 0:1], pchunk[:],
                                     start=True, stop=False)
                    nc.tensor.matmul(piv[:], ct["RED"][:, 1:2],
                                     vns[:, g4:g4 + 4, :], start=False, stop=True)
                    nc.vector.tensor_scalar_min(
                        ivsb[0:1, cc * 512:(cc + 1) * 512], piv[:], 1.0)
                nc.vector.tensor_scalar_max(ivsb[:], ivsb[:], 0.0)
                nc.sync.dma_start(out_t[:], ivsb[:])
            smp_cm.__exit__(None, None, None)

    nc.compile()
    return nc


def host_prepare(d_np, v_np):
    cores = []
    zeros3 = np.zeros((128, 3, 128), np.float32)
    ones3 = np.ones((128, 3, 128), np.float32)
    vext = np.zeros((3, 128, 135, 129), np.float32)
    cm = _const_mats()
    for c in range(8):
        b, hh = c // 2, c % 2
        h0 = 64 * hh
        dpad = np.zeros((128, 70, 128), np.float32)
        lo, hi = h0 - 3, h0 + 67
        src_lo, src_hi = max(lo, 0), min(hi, 128)
        dpad[:, (src_lo - lo):(src_hi - lo), :] = \
            d_np[b, 0, :, src_lo:src_hi, :]
        vext[:] = 0.0
        vext[:, :, 3:131, 0:128] = v_np[b]
        vext[:, :, 131, 0:128] = 2 * v_np[b, :, :, 127, :] - v_np[b, :, :, 126, :]
        vext[:, :, :, 128] = 2 * vext[:, :, :, 127] - vext[:, :, :, 126]
        vin = np.ascontiguousarray(vext[:, :, h0:h0 + 71, :])
        m = {
            "d_in": dpad, "v_in": vin,
            "zpad": np.zeros((128, 64, 6), np.float32),
            "m0_in": zeros3 if hh == 0 else ones3,
            "m1_in": zeros3 if hh == 1 else ones3,
        }
        for name, arr in cm.items():
            m[f"c_{name}"] = arr
        cores.append(m)
    return cores


_NC = None


def kernel(d, v):
    global _NC
    d = np.asarray(d, np.float32)
    v = np.asarray(v, np.float32)
    if _NC is None:
        _NC = build_program()
    in_maps = host_prepare(d, v)
    res = run_bass_kernel_spmd(_NC, in_maps, list(range(8)))
    out = np.zeros((4, 1, 128, 128), np.float32)
    for c in range(8):
        b, hh = c // 2, c % 2
        out[b, 0, 64 * hh:64 * hh + 64, :] = \
            res.results[c]["out"].reshape(64, 128)
    return out



# revision 2
# speedup vs baseline: 1.0418x; 1.0418x over previous
"""Trainium2 Bass kernel for DiffVorticeSketchRender — fp8 DoubleRow version.

Sharding: 8 cores = 4 batches x 2 H-halves (64 rows each + 3-row halos).
Device layout: [D=128 partitions, H slices, W free] everywhere.

Key speedups vs the fp32r baseline:
- v shipped as fp8 hi/lo pairs (hi = e4m3(v), lo = e4m3(v - hi)); every curl
  matmul is a DoubleRow fp8 matmul (0.5 cycles/row) contracting hi and lo in
  one instruction with identical weight rows -> ~bf16 accuracy at 2x speed.
- gaussian smooths: pass1 (D-conv via band matrix fused with H-conv) runs as
  4 DoubleRow matmuls per group, two H-taps per instruction (fp8 weights).
  d-smooth pass2 (W-conv) also fp8 DoubleRow tap pairs; vn-smooth pass2 in
  bf16 (fp8 tap quantization there costs too much accuracy).
- elementwise work spread across Act/DVE/Pool in bf16; transmittance stage
  in bf16 matmuls.
"""

import numpy as np
import ml_dtypes

import concourse.bacc as bacc
import concourse.bass as bass
import concourse.mybir as mybir
import concourse.tile as tile
from bass_rust import AP
from concourse.bass_utils import run_bass_kernel_spmd

F32 = mybir.dt.float32
F8 = mybir.dt.float8e4
BF16 = mybir.dt.bfloat16
AL = mybir.AluOpType
AF = mybir.ActivationFunctionType
DR = mybir.MatmulPerfMode.DoubleRow

E4 = ml_dtypes.float8_e4m3
BF = ml_dtypes.bfloat16

KHS, SIGMA, C = 3, 1.6, 20.0


def _gauss1d():
    size = 2 * KHS + 1
    g = np.arange(size, dtype=np.float64) - (size - 1) / 2.0
    g = np.exp(-((g / SIGMA) ** 2) / 2.0) / (SIGMA * np.sqrt(2.0 * np.pi))
    return (g / g.sum()).astype(np.float32)


GK = _gauss1d()


def _consts():
    eye = np.eye(128, dtype=np.float32)
    mdz = np.zeros((128, 128), np.float32)
    for d in range(127):
        mdz[d, d] = -1.0
        mdz[d, d + 1] = 1.0
    mdz[127, 126] = -1.0
    mdz[127, 127] = 1.0
    mdzt = mdz.T.copy()

    bd = np.zeros((128, 128), np.float32)
    for dp in range(128):
        for k in range(7):
            d = dp + k - 3
            if 0 <= d < 128:
                bd[dp, d] = GK[k]

    # curl DoubleRow weights: identical rows applied to (hi, lo)
    cur = np.zeros((128, 4, 2, 128), np.float32)
    cur[:, 0, 0] = cur[:, 0, 1] = eye
    cur[:, 1, 0] = cur[:, 1, 1] = -eye
    cur[:, 2, 0] = cur[:, 2, 1] = mdzt
    cur[:, 3, 0] = cur[:, 3, 1] = -mdzt

    # pass1 (D+H) tap-pair weights, fp8
    kbd2 = np.zeros((128, 4, 2, 128), np.float32)
    for p in range(4):
        for i in range(2):
            k = 2 * p + i
            if k < 7:
                kbd2[:, p, i] = (GK[k] * bd).T

    # d pass2 (W) tap-pair weights, fp8
    gk8 = GK.astype(E4).astype(np.float32)
    kd2 = np.zeros((128, 4, 2, 128), np.float32)
    for p in range(4):
        for i in range(2):
            k = 2 * p + i
            if k < 7:
                kd2[:, p, i] = gk8[k] * eye

    # vn pass2 taps, bf16
    kwb = np.zeros((128, 7, 128), np.float32)
    for k in range(7):
        kwb[:, k] = GK[k] * eye

    suf = (np.arange(128)[:, None] >= np.arange(128)[None, :]).astype(
        np.float32)
    mc = np.zeros((128, 128), np.float32)
    mc[0, 0], mc[0, 1] = -0.5, 0.5
    for k in range(1, 127):
        mc[k, k - 1], mc[k, k + 1] = -0.5, 0.5
    mc[127, 126], mc[127, 127] = -0.5, -0.5
    sufmct = np.stack([suf, mc.T], axis=1)  # [128, 2, 128]

    red = np.zeros((128, 2), np.float32)
    red[:, 0] = 1.0
    red[127, 1] = 1.0

    return {
        "cur": cur.astype(E4),
        "kbd2": kbd2.astype(E4),
        "kd2": kd2.astype(E4),
        "kwb": kwb.astype(BF),
        "sufmct": sufmct.astype(BF),
        "red": red.astype(BF),
    }


def _wap(t, off, dims):
    """Custom window AP on tile t: free-offset off, free dims [[stride,n]..]."""
    a = t[:]
    return AP(a.tensor, a.offset + off, [list(a.ap[0])] + [list(d) for d in dims])


# vhl free strides
VW, VH, VCH, VHL = 1, 129, 71 * 129, 3 * 71 * 129


def build_program():
    nc = bacc.Bacc("TRN2", target_bir_lowering=False, debug=False)

    vhl_in = nc.dram_tensor("vhl_in", [128, 2, 3, 71, 129], F8,
                            kind="ExternalInput")
    d8_in = nc.dram_tensor("d8_in", [128, 70, 128], F8, kind="ExternalInput")
    # packed constants: c8 = [cur | kbd2 | kd2]; cb = bf16 blob
    # cb layout (free elems): kwb 7*128 | sufmct 2*128 | m0 3*128 | m1 3*128
    # | red 2
    c8_in = nc.dram_tensor("c8_in", [128, 3, 4, 2, 128], F8,
                           kind="ExternalInput")
    cb_in = nc.dram_tensor("cb_in", [128, 1922], BF16, kind="ExternalInput")
    out_t = nc.dram_tensor("out", [1, 8192], F32, kind="ExternalOutput")

    with tile.TileContext(nc) as tc:
        with tc.tile_pool(name="persist", bufs=1) as pp:
            c8 = pp.tile([128, 3, 4, 2, 128], F8, tag="c8")
            cb = pp.tile([128, 1922], BF16, tag="cb")
            cur, kbd2, kd2 = c8[:, 0], c8[:, 1], c8[:, 2]
            kwb = _wap(cb, 0, [[128, 7], [1, 128]])
            sufmct = _wap(cb, 896, [[128, 2], [1, 128]])
            m0t = _wap(cb, 1152, [[128, 3], [1, 128]])
            m1t = _wap(cb, 1536, [[128, 3], [1, 128]])
            red = _wap(cb, 1920, [[1, 2]])
            d8 = pp.tile([128, 71, 128], F8, tag="d8")
            vhl = pp.tile([128, 2, 3, 71, 129], F8, tag="vhl")
            nc.sync.dma_start(c8[:, 0], c8_in[:, 0])
            nc.sync.dma_start(d8[:, 0:12, :], d8_in[:, 0:12, :])
            nc.sync.dma_start(c8[:, 1:3], c8_in[:, 1:3])
            nc.sync.dma_start(cb[:], cb_in[:])
            nc.sync.dma_start(d8[:, 12:70, :], d8_in[:, 12:70, :])
            for a, b in ((0, 12), (12, 24), (24, 36), (36, 48), (48, 60),
                         (60, 71)):
                nc.sync.dma_start(vhl[:, :, :, a:b, :], vhl_in[:, :, :, a:b, :])

            vn8 = pp.tile([128, 71, 128], F8, tag="vn8")
            vn2 = pp.tile([128, 70, 128], BF16, tag="vn2")
            s1d8 = pp.tile([128, 65, 140], F8, tag="s1d8")
            s1b = pp.tile([128, 64, 134], BF16, tag="s1b")
            dsb = pp.tile([128, 64, 128], BF16, tag="dsb")
            vnsb = pp.tile([128, 64, 128], BF16, tag="vnsb")
            ivsb = pp.tile([1, 8192], F32, tag="ivsb")

            # zero the conv pads once (and slack rows read by zero-weight
            # DoubleRow rows)
            nc.gpsimd.memset(d8[:, 70:71, :], 0.0)
            nc.gpsimd.memset(vn8[:, 70:71, :], 0.0)
            nc.gpsimd.memset(s1d8[:, :, 0:3], 0.0)
            nc.gpsimd.memset(s1d8[:, :, 131:140], 0.0)
            nc.gpsimd.memset(s1d8[:, 64:65, :], 0.0)
            nc.gpsimd.memset(s1b[:, :, 0:3], 0.0)
            nc.gpsimd.memset(s1b[:, :, 131:134], 0.0)

            # ---- merged phase: d-smooth (both passes) interleaved with
            # curl. d-smooth is PE-heavy and Act-light; curl is the
            # opposite; interleaving keeps all engines fed. PSUM: shared
            # B/C pool (2 banks) + pcuv/pcw (6 banks) = 8.
            with tc.tile_pool(name="psBC", bufs=2,
                              space=bass.MemorySpace.PSUM) as psBCp, \
                 tc.tile_pool(name="psD", bufs=2,
                              space=bass.MemorySpace.PSUM) as psDp, \
                 tc.tile_pool(name="sqp", bufs=3) as sqp:
                KBD = [_wap(c8, p * 256, [[128, 2], [1, 128]])
                       for p in range(4)]
                KD = [_wap(c8, 2048 + p * 256, [[128, 2], [1, 128]])
                      for p in range(4)]
                IP2 = _wap(c8, 1024 + 0 * 256, [[128, 2], [1, 128]])
                IN2 = _wap(c8, 1024 + 1 * 256, [[128, 2], [1, 128]])
                MDZT2 = _wap(c8, 1024 + 2 * 256, [[128, 2], [1, 128]])
                MDZTN2 = _wap(c8, 1024 + 3 * 256, [[128, 2], [1, 128]])

                def vwin(ch, s, wo, cnt):
                    return _wap(vhl, ch * VCH + s * VH + wo,
                                [[VHL, 2], [VH, cnt], [1, 128]])

                def d_pass1(g4):
                    psB = psBCp.tile([128, 4, 128], F32, tag="psBC")
                    for p in range(4):
                        rhs = _wap(d8, (g4 + 2 * p) * 128,
                                   [[128, 2], [1, 512]])
                        nc.tensor.matmul(psB[:], KBD[p], rhs,
                                         start=(p == 0), stop=(p == 3),
                                         perf_mode=DR)
                    nc.scalar.copy(s1d8[:, g4:g4 + 4, 3:131], psB[:])

                def d_pass2(g4):
                    psC = psBCp.tile([128, 4, 128], F32, tag="psBC")
                    for p in range(4):
                        rhs = _wap(s1d8, g4 * 140 + 2 * p,
                                   [[1, 2], [140, 4], [1, 128]])
                        nc.tensor.matmul(psC[:], KD[p], rhs,
                                         start=(p == 0), stop=(p == 3),
                                         perf_mode=DR)
                    nc.vector.tensor_copy(dsb[:, g4:g4 + 4, :], psC[:])

                def curl_group(s0):
                    cnt = min(4, 70 - s0)
                    pcuv = psDp.tile([128, 2, 4, 128], F32, tag="pcuv")
                    pcw = psDp.tile([128, 4, 128], F32, tag="pcw")
                    ou, ov, ow = (pcuv[:, 0, 0:cnt], pcuv[:, 1, 0:cnt],
                                  pcw[:, 0:cnt])
                    nc.tensor.matmul(ou, IP2, vwin(2, s0 + 1, 0, cnt),
                                     start=True, stop=False, perf_mode=DR)
                    nc.tensor.matmul(ou, IN2, vwin(2, s0, 0, cnt),
                                     start=False, stop=False, perf_mode=DR)
                    nc.tensor.matmul(ou, MDZTN2, vwin(1, s0, 0, cnt),
                                     start=False, stop=True, perf_mode=DR)
                    nc.tensor.matmul(ov, MDZT2, vwin(0, s0, 0, cnt),
                                     start=True, stop=False, perf_mode=DR)
                    nc.tensor.matmul(ov, IN2, vwin(2, s0, 1, cnt),
                                     start=False, stop=False, perf_mode=DR)
                    nc.tensor.matmul(ov, IP2, vwin(2, s0, 0, cnt),
                                     start=False, stop=True, perf_mode=DR)

                    nc.tensor.matmul(ow, IP2, vwin(1, s0, 1, cnt),
                                     start=True, stop=False, perf_mode=DR)
                    nc.tensor.matmul(ow, IN2, vwin(1, s0, 0, cnt),
                                     start=False, stop=False, perf_mode=DR)
                    nc.tensor.matmul(ow, IN2, vwin(0, s0 + 1, 0, cnt),
                                     start=False, stop=False, perf_mode=DR)
                    nc.tensor.matmul(ow, IP2, vwin(0, s0, 0, cnt),
                                     start=False, stop=True, perf_mode=DR)

                    squv = sqp.tile([128, 2, 4, 128], BF16, tag="squv")
                    cwb = sqp.tile([128, 4, 128], BF16, tag="cwb")
                    sqw = sqp.tile([128, 4, 128], BF16, tag="sqw")
                    tsum = sqp.tile([128, 4, 128], BF16, tag="tsum")
                    au, av = squv[:, 0, 0:cnt], squv[:, 1, 0:cnt]
                    acw, aw = cwb[:, 0:cnt], sqw[:, 0:cnt]
                    ats = tsum[:, 0:cnt]
                    vn2g = vn2[:, s0:s0 + cnt, :]
                    nc.scalar.activation(squv[:, :, 0:cnt], pcuv[:, :, 0:cnt],
                                         AF.Square)
                    nc.vector.tensor_copy(acw, ow)
                    nc.vector.tensor_mul(aw, acw, acw)
                    nc.vector.tensor_add(ats, au, av)
                    nc.gpsimd.tensor_add(vn2g, ats, aw)
                    if s0 == 0:
                        nc.vector.tensor_mul(vn2[:, 0:3], vn2[:, 0:3], m0t)
                    if s0 <= 67 < s0 + cnt:
                        nc.vector.tensor_mul(
                            vn2[:, 67:s0 + cnt], vn2[:, 67:s0 + cnt],
                            _wap(cb, 1536, [[128, s0 + cnt - 67], [1, 128]]))
                    elif s0 > 67:
                        nc.vector.tensor_mul(
                            vn2[:, s0:s0 + cnt], vn2[:, s0:s0 + cnt],
                            _wap(cb, 1536 + (s0 - 67) * 128,
                                 [[128, cnt], [1, 128]]))
                    return s0 + cnt

                # note curl groups: s0 = 0,4,...,64 (17 groups of 4) + 68 (2)
                d_pass1(0)
                d_pass1(4)
                d_pass1(8)
                d_pass2(0)
                s0 = 0
                sqrt_done = 0
                for it in range(18):
                    s0 = curl_group(s0)
                    if it + 3 <= 15:
                        d_pass1((it + 3) * 4)
                    if it + 1 <= 15:
                        d_pass2((it + 1) * 4)
                    while sqrt_done + 14 <= s0 and (s0 >= 70 or
                                                    sqrt_done + 14 <= s0 - 3):
                        a = sqrt_done
                        nc.scalar.activation(vn8[:, a:a + 14, :],
                                             vn2[:, a:a + 14, :], AF.Sqrt)
                        sqrt_done += 14

            # ---- phase E: vn-smooth + transmittance integration ----
            # PE stream per iteration: p1(cc) SUF(cc) p2(cc) MCT(cc-1)
            # RED(cc-2): the exp/bcb/tb chain for group cc gets a full
            # iteration before MCT consumes tb, and pch gets another before
            # RED consumes it, so the PE never waits on elementwise.
            with tc.tile_pool(name="psA", bufs=2,
                              space=bass.MemorySpace.PSUM) as psAp, \
                 tc.tile_pool(name="psW", bufs=2,
                              space=bass.MemorySpace.PSUM) as psWp, \
                 tc.tile_pool(name="ps4", bufs=1,
                              space=bass.MemorySpace.PSUM) as ps4p, \
                 tc.tile_pool(name="psM", bufs=2,
                              space=bass.MemorySpace.PSUM) as psMp, \
                 tc.tile_pool(name="piv", bufs=1,
                              space=bass.MemorySpace.PSUM) as pivp, \
                 tc.tile_pool(name="st4", bufs=3) as st4:
                KW = [_wap(cb, k * 128, [[1, 128]]) for k in range(7)]
                SUFW = _wap(cb, 896, [[1, 128]])
                MCTW = _wap(cb, 1024, [[1, 128]])
                RED0 = _wap(cb, 1920, [[1, 1]])
                RED1 = _wap(cb, 1921, [[1, 1]])
                mct_q = []   # (g4, tb)
                red_q = []   # (g4, pch, cc)

                def do_mct(g4, tb):
                    psM = psMp.tile([128, 4, 128], F32, tag="psM")
                    nc.tensor.matmul(psM[:], MCTW, tb[:],
                                     start=True, stop=True)
                    pch = st4.tile([128, 4, 128], BF16, tag="pch")
                    nc.vector.tensor_mul(pch[:], psM[:],
                                         vnsb[:, g4:g4 + 4, :])
                    return pch

                def do_red(g4, pch, pcc):
                    pv = pivp.tile([1, 512], F32, tag="pv")
                    nc.tensor.matmul(pv[:], RED0, pch[:],
                                     start=True, stop=False)
                    nc.tensor.matmul(pv[:], RED1, vnsb[:, g4:g4 + 4, :],
                                     start=False, stop=True)
                    nc.vector.tensor_scalar(
                        ivsb[0:1, pcc * 512:(pcc + 1) * 512], pv[:],
                        1.0, 0.0, AL.min, AL.max)

                for cc in range(16):
                    g4 = cc * 4
                    psA = psAp.tile([128, 4, 128], F32, tag="psA")
                    for p in range(4):
                        rhs = _wap(vn8, (g4 + 2 * p) * 128,
                                   [[128, 2], [1, 512]])
                        lhsT = _wap(c8, p * 256, [[128, 2], [1, 128]])
                        nc.tensor.matmul(psA[:], lhsT, rhs,
                                         start=(p == 0), stop=(p == 3),
                                         perf_mode=DR)
                    ps4 = ps4p.tile([128, 4, 128], F32, tag="ps4")
                    nc.tensor.matmul(ps4[:], SUFW, dsb[:, g4:g4 + 4, :],
                                     start=True, stop=True)
                    nc.scalar.copy(s1b[:, g4:g4 + 4, 3:131], psA[:])
                    ecb = st4.tile([128, 4, 128], BF16, tag="ecb")
                    bcb = st4.tile([128, 4, 128], BF16, tag="bcb")
                    tb = st4.tile([128, 4, 128], BF16, tag="tb")
                    nc.scalar.activation(ecb[:], ps4[:], AF.Exp, scale=-C)
                    nc.vector.tensor_scalar(bcb[:], ps4[:], C, 1.0,
                                            AL.mult, AL.add)
                    nc.gpsimd.tensor_mul(tb[:], bcb[:], ecb[:])
                    psW = psWp.tile([128, 4, 128], F32, tag="psW")
                    for k in range(7):
                        nc.tensor.matmul(psW[:], KW[k],
                                         s1b[:, g4:g4 + 4, k:k + 128],
                                         start=(k == 0), stop=(k == 6))
                    nc.scalar.copy(vnsb[:, g4:g4 + 4, :], psW[:])
                    if mct_q:
                        pg4, ptb = mct_q.pop(0)
                        pch = do_mct(pg4, ptb)
                        red_q.append((pg4, pch, cc - 1))
                    if len(red_q) > 1:
                        do_red(*red_q.pop(0))
                    mct_q.append((g4, tb))
                while mct_q:
                    pg4, ptb = mct_q.pop(0)
                    pch = do_mct(pg4, ptb)
                    red_q.append((pg4, pch, 15))
                while red_q:
                    do_red(*red_q.pop(0))
                nc.sync.dma_start(out_t[:], ivsb[:])

    nc.compile()
    return nc


def host_prepare(d_np, v_np):
    cm = _consts()
    c8 = np.stack([cm["kbd2"].astype(np.float32),
                   cm["cur"].astype(np.float32),
                   cm["kd2"].astype(np.float32)], axis=1).astype(E4)
    zeros3 = np.zeros((128, 3, 128), np.float32)
    ones3 = np.ones((128, 3, 128), np.float32)

    def cb_blob(hh):
        m0 = zeros3 if hh == 0 else ones3
        m1 = zeros3 if hh == 1 else ones3
        parts = [cm["kwb"].astype(np.float32).reshape(128, -1),
                 cm["sufmct"].astype(np.float32).reshape(128, -1),
                 m0.reshape(128, -1), m1.reshape(128, -1),
                 cm["red"].astype(np.float32).reshape(128, -1)]
        return np.concatenate(parts, axis=1).astype(BF)

    cb0, cb1 = cb_blob(0), cb_blob(1)
    cores = []
    vext = np.zeros((3, 128, 135, 129), np.float32)
    for c in range(8):
        b, hh = c // 2, c % 2
        h0 = 64 * hh
        dpad = np.zeros((128, 70, 128), np.float32)
        lo, hi = h0 - 3, h0 + 67
        src_lo, src_hi = max(lo, 0), min(hi, 128)
        dpad[:, (src_lo - lo):(src_hi - lo), :] = \
            d_np[b, 0, :, src_lo:src_hi, :]
        vext[:] = 0.0
        vext[:, :, 3:131, 0:128] = v_np[b]
        vext[:, :, 131, 0:128] = \
            2 * v_np[b, :, :, 127, :] - v_np[b, :, :, 126, :]
        vext[:, :, :, 128] = 2 * vext[:, :, :, 127] - vext[:, :, :, 126]
        vin = vext[:, :, h0:h0 + 71, :]  # [3, 128, 71, 129]
        vhi = vin.astype(E4)
        vlo = (vin - vhi.astype(np.float32)).astype(E4)
        vhl = np.stack([vhi, vlo], axis=0)  # [2, 3, 128, 71, 129]
        vhl = np.ascontiguousarray(vhl.transpose(2, 0, 1, 3, 4))
        m = {
            "vhl_in": vhl,
            "d8_in": dpad.astype(E4),
            "c8_in": c8,
            "cb_in": cb0 if hh == 0 else cb1,
        }
        cores.append(m)
    return cores


_NC = None


def kernel(d, v):
    global _NC
    d = np.asarray(d, np.float32)
    v = np.asarray(v, np.float32)
    if _NC is None:
        _NC = build_program()
    in_maps = host_prepare(d, v)
    res = run_bass_kernel_spmd(_NC, in_maps, list(range(8)))
    out = np.zeros((4, 1, 128, 128), np.float32)
    for c in range(8):
        b, hh = c // 2, c % 2
        out[b, 0, 64 * hh:64 * hh + 64, :] = \
            res.results[c]["out"].reshape(64, 128)
    return out


# revision 5
# speedup vs baseline: 1.0754x; 1.0322x over previous
"""Trainium2 Bass kernel for DiffVorticeSketchRender — fp8 DoubleRow version.

Sharding: 8 cores = 4 batches x 2 H-halves (64 rows each + 3-row halos).
Device layout: [D=128 partitions, H slices, W free] everywhere.

Key speedups vs the fp32r baseline:
- v shipped as fp8 hi/lo pairs (hi = e4m3(v), lo = e4m3(v - hi)); every curl
  matmul is a DoubleRow fp8 matmul (0.5 cycles/row) contracting hi and lo in
  one instruction with identical weight rows -> ~bf16 accuracy at 2x speed.
- gaussian smooths: pass1 (D-conv via band matrix fused with H-conv) runs as
  4 DoubleRow matmuls per group, two H-taps per instruction (fp8 weights).
  d-smooth pass2 (W-conv) also fp8 DoubleRow tap pairs; vn-smooth pass2 in
  bf16 (fp8 tap quantization there costs too much accuracy).
- elementwise work spread across Act/DVE/Pool in bf16; transmittance stage
  in bf16 matmuls.
"""

import numpy as np
import ml_dtypes

import concourse.bacc as bacc
import concourse.bass as bass
import concourse.mybir as mybir
import concourse.tile as tile
from bass_rust import AP
from concourse.bass_utils import run_bass_kernel_spmd

F32 = mybir.dt.float32
F8 = mybir.dt.float8e4
BF16 = mybir.dt.bfloat16
AL = mybir.AluOpType
AF = mybir.ActivationFunctionType
DR = mybir.MatmulPerfMode.DoubleRow

E4 = ml_dtypes.float8_e4m3
BF = ml_dtypes.bfloat16

KHS, SIGMA, C = 3, 1.6, 20.0


def _gauss1d():
    size = 2 * KHS + 1
    g = np.arange(size, dtype=np.float64) - (size - 1) / 2.0
    g = np.exp(-((g / SIGMA) ** 2) / 2.0) / (SIGMA * np.sqrt(2.0 * np.pi))
    return (g / g.sum()).astype(np.float32)


GK = _gauss1d()


def _consts():
    eye = np.eye(128, dtype=np.float32)
    mdz = np.zeros((128, 128), np.float32)
    for d in range(127):
        mdz[d, d] = -1.0
        mdz[d, d + 1] = 1.0
    mdz[127, 126] = -1.0
    mdz[127, 127] = 1.0
    mdzt = mdz.T.copy()

    bd = np.zeros((128, 128), np.float32)
    for dp in range(128):
        for k in range(7):
            d = dp + k - 3
            if 0 <= d < 128:
                bd[dp, d] = GK[k]

    # curl DoubleRow weights: identical rows applied to (hi, lo)
    cur = np.zeros((128, 4, 2, 128), np.float32)
    cur[:, 0, 0] = cur[:, 0, 1] = eye
    cur[:, 1, 0] = cur[:, 1, 1] = -eye
    cur[:, 2, 0] = cur[:, 2, 1] = mdzt
    cur[:, 3, 0] = cur[:, 3, 1] = -mdzt

    # pass1 (D+H) tap-pair weights, fp8
    kbd2 = np.zeros((128, 4, 2, 128), np.float32)
    for p in range(4):
        for i in range(2):
            k = 2 * p + i
            if k < 7:
                kbd2[:, p, i] = (GK[k] * bd).T

    # d pass2 (W) tap-pair weights, fp8
    gk8 = GK.astype(E4).astype(np.float32)
    kd2 = np.zeros((128, 4, 2, 128), np.float32)
    for p in range(4):
        for i in range(2):
            k = 2 * p + i
            if k < 7:
                kd2[:, p, i] = gk8[k] * eye

    # vn pass2 taps, bf16
    kwb = np.zeros((128, 7, 128), np.float32)
    for k in range(7):
        kwb[:, k] = GK[k] * eye

    suf = (np.arange(128)[:, None] >= np.arange(128)[None, :]).astype(
        np.float32)
    mc = np.zeros((128, 128), np.float32)
    mc[0, 0], mc[0, 1] = -0.5, 0.5
    for k in range(1, 127):
        mc[k, k - 1], mc[k, k + 1] = -0.5, 0.5
    mc[127, 126], mc[127, 127] = -0.5, -0.5
    sufmct = np.stack([suf, mc.T], axis=1)  # [128, 2, 128]

    red = np.zeros((128, 2), np.float32)
    red[:, 0] = 1.0
    red[127, 1] = 1.0

    return {
        "cur": cur.astype(E4),
        "kbd2": kbd2.astype(E4),
        "kd2": kd2.astype(E4),
        "kwb": kwb.astype(BF),
        "sufmct": sufmct.astype(BF),
        "red": red.astype(BF),
    }


def _wap(t, off, dims):
    """Custom window AP on tile t: free-offset off, free dims [[stride,n]..]."""
    a = t[:]
    return AP(a.tensor, a.offset + off, [list(a.ap[0])] + [list(d) for d in dims])


# vhl free strides
VW, VH, VCH, VHL = 1, 129, 71 * 129, 3 * 71 * 129


def build_program():
    nc = bacc.Bacc("TRN2", target_bir_lowering=False, debug=False)

    vhl_in = nc.dram_tensor("vhl_in", [128, 2, 3, 71, 129], F8,
                            kind="ExternalInput")
    d8_in = nc.dram_tensor("d8_in", [128, 70, 128], F8, kind="ExternalInput")
    # packed constants: c8 = [cur | kbd2 | kd2]; cb = bf16 blob
    # cb layout (free elems): kwb 7*128 | sufmct 2*128 | m0 3*128 | m1 3*128
    # | red 2
    c8_in = nc.dram_tensor("c8_in", [128, 3, 4, 2, 128], F8,
                           kind="ExternalInput")
    cb_in = nc.dram_tensor("cb_in", [128, 1922], BF16, kind="ExternalInput")
    out_t = nc.dram_tensor("out", [1, 8192], F32, kind="ExternalOutput")

    with tile.TileContext(nc) as tc:
        with tc.tile_pool(name="persist", bufs=1) as pp:
            c8 = pp.tile([128, 3, 4, 2, 128], F8, tag="c8")
            cb = pp.tile([128, 1922], BF16, tag="cb")
            cur, kbd2, kd2 = c8[:, 0], c8[:, 1], c8[:, 2]
            kwb = _wap(cb, 0, [[128, 7], [1, 128]])
            sufmct = _wap(cb, 896, [[128, 2], [1, 128]])
            m0t = _wap(cb, 1152, [[128, 3], [1, 128]])
            m1t = _wap(cb, 1536, [[128, 3], [1, 128]])
            red = _wap(cb, 1920, [[1, 2]])
            d8 = pp.tile([128, 71, 128], F8, tag="d8")
            vhl = pp.tile([128, 2, 3, 71, 129], F8, tag="vhl")
            nc.sync.dma_start(c8[:, 0], c8_in[:, 0])
            nc.sync.dma_start(d8[:, 0:14, :], d8_in[:, 0:14, :])
            nc.sync.dma_start(vhl[:, :, :, 0:6, :], vhl_in[:, :, :, 0:6, :])
            nc.sync.dma_start(c8[:, 1:3], c8_in[:, 1:3])
            nc.sync.dma_start(vhl[:, :, :, 6:18, :], vhl_in[:, :, :, 6:18, :])
            nc.sync.dma_start(d8[:, 14:70, :], d8_in[:, 14:70, :])
            nc.sync.dma_start(cb[:], cb_in[:])
            for a, b in ((18, 30), (30, 42), (42, 54), (54, 66), (66, 71)):
                nc.sync.dma_start(vhl[:, :, :, a:b, :], vhl_in[:, :, :, a:b, :])

            vn8 = pp.tile([128, 71, 128], F8, tag="vn8")
            vn2 = pp.tile([128, 70, 128], BF16, tag="vn2")
            s1d8 = pp.tile([128, 65, 140], F8, tag="s1d8")
            s1b = pp.tile([128, 64, 134], BF16, tag="s1b")
            dsb = pp.tile([128, 64, 128], BF16, tag="dsb")
            vnsb = pp.tile([128, 64, 128], BF16, tag="vnsb")
            ivsb = pp.tile([1, 8192], F32, tag="ivsb")

            # zero the conv pads once (and slack rows read by zero-weight
            # DoubleRow rows)
            nc.gpsimd.memset(d8[:, 70:71, :], 0.0)
            nc.gpsimd.memset(vn8[:, 70:71, :], 0.0)
            nc.gpsimd.memset(s1d8[:, :, 0:3], 0.0)
            nc.gpsimd.memset(s1d8[:, :, 131:140], 0.0)
            nc.gpsimd.memset(s1d8[:, 64:65, :], 0.0)
            nc.gpsimd.memset(s1b[:, :, 0:3], 0.0)
            nc.gpsimd.memset(s1b[:, :, 131:134], 0.0)

            # ---- merged phase: d-smooth (both passes) interleaved with
            # curl. d-smooth is PE-heavy and Act-light; curl is the
            # opposite; interleaving keeps all engines fed. PSUM: shared
            # B/C pool (2 banks) + pcuv/pcw (6 banks) = 8.
            with tc.tile_pool(name="psBC", bufs=2,
                              space=bass.MemorySpace.PSUM) as psBCp, \
                 tc.tile_pool(name="psD", bufs=2,
                              space=bass.MemorySpace.PSUM) as psDp, \
                 tc.tile_pool(name="sqp", bufs=3) as sqp:
                KBD = [_wap(c8, p * 256, [[128, 2], [1, 128]])
                       for p in range(4)]
                KD = [_wap(c8, 2048 + p * 256, [[128, 2], [1, 128]])
                      for p in range(4)]
                IP2 = _wap(c8, 1024 + 0 * 256, [[128, 2], [1, 128]])
                IN2 = _wap(c8, 1024 + 1 * 256, [[128, 2], [1, 128]])
                MDZT2 = _wap(c8, 1024 + 2 * 256, [[128, 2], [1, 128]])
                MDZTN2 = _wap(c8, 1024 + 3 * 256, [[128, 2], [1, 128]])

                def vwin(ch, s, wo, cnt):
                    return _wap(vhl, ch * VCH + s * VH + wo,
                                [[VHL, 2], [VH, cnt], [1, 128]])

                def d_pass1(g4):
                    psB = psBCp.tile([128, 4, 128], F32, tag="psBC")
                    for p in range(4):
                        rhs = _wap(d8, (g4 + 2 * p) * 128,
                                   [[128, 2], [1, 512]])
                        nc.tensor.matmul(psB[:], KBD[p], rhs,
                                         start=(p == 0), stop=(p == 3),
                                         perf_mode=DR)
                    nc.scalar.copy(s1d8[:, g4:g4 + 4, 3:131], psB[:])

                def d_pass2(g4):
                    psC = psBCp.tile([128, 4, 128], F32, tag="psBC")
                    for p in range(4):
                        rhs = _wap(s1d8, g4 * 140 + 2 * p,
                                   [[1, 2], [140, 4], [1, 128]])
                        nc.tensor.matmul(psC[:], KD[p], rhs,
                                         start=(p == 0), stop=(p == 3),
                                         perf_mode=DR)
                    nc.vector.tensor_copy(dsb[:, g4:g4 + 4, :], psC[:])

                def curl_group(s0):
                    cnt = min(4, 70 - s0)
                    pcuv = psDp.tile([128, 2, 4, 128], F32, tag="pcuv")
                    pcw = psDp.tile([128, 4, 128], F32, tag="pcw")
                    ou, ov, ow = (pcuv[:, 0, 0:cnt], pcuv[:, 1, 0:cnt],
                                  pcw[:, 0:cnt])
                    nc.tensor.matmul(ou, IP2, vwin(2, s0 + 1, 0, cnt),
                                     start=True, stop=False, perf_mode=DR)
                    nc.tensor.matmul(ou, IN2, vwin(2, s0, 0, cnt),
                                     start=False, stop=False, perf_mode=DR)
                    nc.tensor.matmul(ou, MDZTN2, vwin(1, s0, 0, cnt),
                                     start=False, stop=True, perf_mode=DR)
                    nc.tensor.matmul(ov, MDZT2, vwin(0, s0, 0, cnt),
                                     start=True, stop=False, perf_mode=DR)
                    nc.tensor.matmul(ov, IN2, vwin(2, s0, 1, cnt),
                                     start=False, stop=False, perf_mode=DR)
                    nc.tensor.matmul(ov, IP2, vwin(2, s0, 0, cnt),
                                     start=False, stop=True, perf_mode=DR)

                    nc.tensor.matmul(ow, IP2, vwin(1, s0, 1, cnt),
                                     start=True, stop=False, perf_mode=DR)
                    nc.tensor.matmul(ow, IN2, vwin(1, s0, 0, cnt),
                                     start=False, stop=False, perf_mode=DR)
                    nc.tensor.matmul(ow, IN2, vwin(0, s0 + 1, 0, cnt),
                                     start=False, stop=False, perf_mode=DR)
                    nc.tensor.matmul(ow, IP2, vwin(0, s0, 0, cnt),
                                     start=False, stop=True, perf_mode=DR)

                    squv = sqp.tile([128, 2, 4, 128], BF16, tag="squv")
                    cwb = sqp.tile([128, 4, 128], BF16, tag="cwb")
                    sqw = sqp.tile([128, 4, 128], BF16, tag="sqw")
                    tsum = sqp.tile([128, 4, 128], BF16, tag="tsum")
                    au, av = squv[:, 0, 0:cnt], squv[:, 1, 0:cnt]
                    acw, aw = cwb[:, 0:cnt], sqw[:, 0:cnt]
                    ats = tsum[:, 0:cnt]
                    vn2g = vn2[:, s0:s0 + cnt, :]
                    nc.scalar.activation(squv[:, :, 0:cnt], pcuv[:, :, 0:cnt],
                                         AF.Square)
                    nc.vector.tensor_copy(acw, ow)
                    nc.vector.tensor_mul(aw, acw, acw)
                    nc.vector.tensor_add(ats, au, av)
                    nc.gpsimd.tensor_add(vn2g, ats, aw)
                    if s0 == 0:
                        nc.vector.tensor_mul(vn2[:, 0:3], vn2[:, 0:3], m0t)
                    if s0 <= 67 < s0 + cnt:
                        nc.vector.tensor_mul(
                            vn2[:, 67:s0 + cnt], vn2[:, 67:s0 + cnt],
                            _wap(cb, 1536, [[128, s0 + cnt - 67], [1, 128]]))
                    elif s0 > 67:
                        nc.vector.tensor_mul(
                            vn2[:, s0:s0 + cnt], vn2[:, s0:s0 + cnt],
                            _wap(cb, 1536 + (s0 - 67) * 128,
                                 [[128, cnt], [1, 128]]))
                    return s0 + cnt

                # note curl groups: s0 = 0,4,...,64 (17 groups of 4) + 68 (2)
                # front-load d-smooth so the PE queue never blocks behind the
                # first curl group's v DMA
                for g in range(2):
                    d_pass1(g * 4)
                d_pass2(0)
                def e_pre(cc):
                    # pre-run vn-smooth pass1 for the first E groups out of
                    # the (drained) B/C psum pool so their s1b copies queue
                    # on Act before the sqrt->exp table switch
                    psE = psBCp.tile([128, 4, 128], F32, tag="psBC")
                    g4 = cc * 4
                    for p in range(4):
                        rhs = _wap(vn8, (g4 + 2 * p) * 128,
                                   [[128, 2], [1, 512]])
                        nc.tensor.matmul(psE[:], KBD[p], rhs,
                                         start=(p == 0), stop=(p == 3),
                                         perf_mode=DR)
                    nc.scalar.copy(s1b[:, g4:g4 + 4, 3:131], psE[:])

                s0 = 0
                sqrt_chunks = [(0, 14), (14, 28), (28, 42), (42, 56),
                               (56, 66), (66, 70)]
                for it in range(18):
                    s0 = curl_group(s0)
                    if it + 2 <= 15:
                        d_pass1((it + 2) * 4)
                    if it + 1 <= 15:
                        d_pass2((it + 1) * 4)
                    while sqrt_chunks and sqrt_chunks[0][1] <= s0:
                        a, b = sqrt_chunks.pop(0)
                        nc.scalar.activation(vn8[:, a:b, :],
                                             vn2[:, a:b, :], AF.Sqrt)
                e_pre(0)
                e_pre(1)

            # ---- phase E: vn-smooth + transmittance integration ----
            # PE stream per iteration: p1(cc) SUF(cc) p2(cc) MCT(cc-1)
            # RED(cc-2): the exp/bcb/tb chain for group cc gets a full
            # iteration before MCT consumes tb, and pch gets another before
            # RED consumes it, so the PE never waits on elementwise.
            with tc.tile_pool(name="psA", bufs=2,
                              space=bass.MemorySpace.PSUM) as psAp, \
                 tc.tile_pool(name="psW", bufs=1,
                              space=bass.MemorySpace.PSUM) as psWp, \
                 tc.tile_pool(name="ps4", bufs=1,
                              space=bass.MemorySpace.PSUM) as ps4p, \
                 tc.tile_pool(name="psM", bufs=2,
                              space=bass.MemorySpace.PSUM) as psMp, \
                 tc.tile_pool(name="piv", bufs=2,
                              space=bass.MemorySpace.PSUM) as pivp, \
                 tc.tile_pool(name="st4", bufs=3) as st4:
                KW = [_wap(cb, k * 128, [[1, 128]]) for k in range(7)]
                SUFW = _wap(cb, 896, [[1, 128]])
                MCTW = _wap(cb, 1024, [[1, 128]])
                RED0 = _wap(cb, 1920, [[1, 1]])
                RED1 = _wap(cb, 1921, [[1, 1]])
                mct_q = []   # (g4, tb)
                red_q = []   # (g4, pch, cc)

                def do_mct(g4, tb):
                    psM = psMp.tile([128, 4, 128], F32, tag="psM")
                    nc.tensor.matmul(psM[:], MCTW, tb[:],
                                     start=True, stop=True)
                    pch = st4.tile([128, 4, 128], BF16, tag="pch")
                    nc.vector.tensor_mul(pch[:], psM[:],
                                         vnsb[:, g4:g4 + 4, :])
                    return pch

                def do_red(g4, pch, pcc):
                    pv = pivp.tile([1, 512], F32, tag="pv")
                    nc.tensor.matmul(pv[:], RED0, pch[:],
                                     start=True, stop=False)
                    nc.tensor.matmul(pv[:], RED1, vnsb[:, g4:g4 + 4, :],
                                     start=False, stop=True)
                    nc.vector.tensor_scalar(
                        ivsb[0:1, pcc * 512:(pcc + 1) * 512], pv[:],
                        1.0, 0.0, AL.min, AL.max)
                    if pcc == 7:
                        nc.sync.dma_start(out_t[0:1, 0:4096],
                                          ivsb[0:1, 0:4096])

                def do_p2(g4):
                    psW = psWp.tile([128, 4, 128], F32, tag="psW")
                    for k in range(7):
                        nc.tensor.matmul(psW[:], KW[k],
                                         s1b[:, g4:g4 + 4, k:k + 128],
                                         start=(k == 0), stop=(k == 6))
                    nc.scalar.copy(vnsb[:, g4:g4 + 4, :], psW[:])

                for cc in range(16):
                    g4 = cc * 4
                    if cc >= 2:
                        psA = psAp.tile([128, 4, 128], F32, tag="psA")
                        for p in range(4):
                            rhs = _wap(vn8, (g4 + 2 * p) * 128,
                                       [[128, 2], [1, 512]])
                            lhsT = _wap(c8, p * 256,
                                        [[128, 2], [1, 128]])
                            nc.tensor.matmul(psA[:], lhsT, rhs,
                                             start=(p == 0), stop=(p == 3),
                                             perf_mode=DR)
                    ps4 = ps4p.tile([128, 4, 128], F32, tag="ps4")
                    nc.tensor.matmul(ps4[:], SUFW, dsb[:, g4:g4 + 4, :],
                                     start=True, stop=True)
                    if cc >= 2:
                        nc.scalar.copy(s1b[:, g4:g4 + 4, 3:131], psA[:])
                    ecb = st4.tile([128, 4, 128], BF16, tag="ecb")
                    bcb = st4.tile([128, 4, 128], BF16, tag="bcb")
                    tb = st4.tile([128, 4, 128], BF16, tag="tb")
                    nc.scalar.activation(ecb[:], ps4[:], AF.Exp, scale=-C)
                    nc.vector.tensor_scalar(bcb[:], ps4[:], C, 1.0,
                                            AL.mult, AL.add)
                    nc.gpsimd.tensor_mul(tb[:], bcb[:], ecb[:])
                    if cc >= 1:
                        do_p2(g4 - 4)
                    if len(mct_q) > 1:
                        pg4, ptb = mct_q.pop(0)
                        pch = do_mct(pg4, ptb)
                        red_q.append((pg4, pch, pg4 // 4))
                    if len(red_q) > 1:
                        do_red(*red_q.pop(0))
                    mct_q.append((g4, tb))
                do_p2(60)
                while mct_q:
                    pg4, ptb = mct_q.pop(0)
                    pch = do_mct(pg4, ptb)
                    red_q.append((pg4, pch, pg4 // 4))
                while red_q:
                    do_red(*red_q.pop(0))
                nc.sync.dma_start(out_t[0:1, 4096:8192],
                                  ivsb[0:1, 4096:8192])

    nc.compile()
    return nc


def host_prepare(d_np, v_np):
    cm = _consts()
    c8 = np.stack([cm["kbd2"].astype(np.float32),
                   cm["cur"].astype(np.float32),
                   cm["kd2"].astype(np.float32)], axis=1).astype(E4)
    zeros3 = np.zeros((128, 3, 128), np.float32)
    ones3 = np.ones((128, 3, 128), np.float32)

    def cb_blob(hh):
        m0 = zeros3 if hh == 0 else ones3
        m1 = zeros3 if hh == 1 else ones3
        parts = [cm["kwb"].astype(np.float32).reshape(128, -1),
                 cm["sufmct"].astype(np.float32).reshape(128, -1),
                 m0.reshape(128, -1), m1.reshape(128, -1),
                 cm["red"].astype(np.float32).reshape(128, -1)]
        return np.concatenate(parts, axis=1).astype(BF)

    cb0, cb1 = cb_blob(0), cb_blob(1)
    cores = []
    vext = np.zeros((3, 128, 135, 129), np.float32)
    for c in range(8):
        b, hh = c // 2, c % 2
        h0 = 64 * hh
        dpad = np.zeros((128, 70, 128), np.float32)
        lo, hi = h0 - 3, h0 + 67
        src_lo, src_hi = max(lo, 0), min(hi, 128)
        dpad[:, (src_lo - lo):(src_hi - lo), :] = \
            d_np[b, 0, :, src_lo:src_hi, :]
        vext[:] = 0.0
        vext[:, :, 3:131, 0:128] = v_np[b]
        vext[:, :, 131, 0:128] = \
            2 * v_np[b, :, :, 127, :] - v_np[b, :, :, 126, :]
        vext[:, :, :, 128] = 2 * vext[:, :, :, 127] - vext[:, :, :, 126]
        vin = vext[:, :, h0:h0 + 71, :]  # [3, 128, 71, 129]
        vhi = vin.astype(E4)
        vlo = (vin - vhi.astype(np.float32)).astype(E4)
        vhl = np.stack([vhi, vlo], axis=0)  # [2, 3, 128, 71, 129]
        vhl = np.ascontiguousarray(vhl.transpose(2, 0, 1, 3, 4))
        m = {
            "vhl_in": vhl,
            "d8_in": dpad.astype(E4),
            "c8_in": c8,
            "cb_in": cb0 if hh == 0 else cb1,
        }
        cores.append(m)
    return cores


_NC = None


def kernel(d, v):
    global _NC
    d = np.asarray(d, np.float32)
    v = np.asarray(v, np.float32)
    if _NC is None:
        _NC = build_program()
    in_maps = host_prepare(d, v)
    res = run_bass_kernel_spmd(_NC, in_maps, list(range(8)))
    out = np.zeros((4, 1, 128, 128), np.float32)
    for c in range(8):
        b, hh = c // 2, c % 2
        out[b, 0, 64 * hh:64 * hh + 64, :] = \
            res.results[c]["out"].reshape(64, 128)
    return out


# revision 6
# speedup vs baseline: 1.0762x; 1.0008x over previous
"""Trainium2 Bass kernel for DiffVorticeSketchRender — fp8 DoubleRow version.

Sharding: 8 cores = 4 batches x 2 H-halves (64 rows each + 3-row halos).
Device layout: [D=128 partitions, H slices, W free] everywhere.

Key speedups vs the fp32r baseline:
- v shipped as fp8 hi/lo pairs (hi = e4m3(v), lo = e4m3(v - hi)); every curl
  matmul is a DoubleRow fp8 matmul (0.5 cycles/row) contracting hi and lo in
  one instruction with identical weight rows -> ~bf16 accuracy at 2x speed.
- gaussian smooths: pass1 (D-conv via band matrix fused with H-conv) runs as
  4 DoubleRow matmuls per group, two H-taps per instruction (fp8 weights).
  d-smooth pass2 (W-conv) also fp8 DoubleRow tap pairs; vn-smooth pass2 in
  bf16 (fp8 tap quantization there costs too much accuracy).
- elementwise work spread across Act/DVE/Pool in bf16; transmittance stage
  in bf16 matmuls.
"""

import numpy as np
import ml_dtypes

import concourse.bacc as bacc
import concourse.bass as bass
import concourse.mybir as mybir
import concourse.tile as tile
from bass_rust import AP
from concourse.bass_utils import run_bass_kernel_spmd

F32 = mybir.dt.float32
F8 = mybir.dt.float8e4
BF16 = mybir.dt.bfloat16
AL = mybir.AluOpType
AF = mybir.ActivationFunctionType
DR = mybir.MatmulPerfMode.DoubleRow

E4 = ml_dtypes.float8_e4m3
BF = ml_dtypes.bfloat16

KHS, SIGMA, C = 3, 1.6, 20.0


def _gauss1d():
    size = 2 * KHS + 1
    g = np.arange(size, dtype=np.float64) - (size - 1) / 2.0
    g = np.exp(-((g / SIGMA) ** 2) / 2.0) / (SIGMA * np.sqrt(2.0 * np.pi))
    return (g / g.sum()).astype(np.float32)


GK = _gauss1d()


def _consts():
    eye = np.eye(128, dtype=np.float32)
    mdz = np.zeros((128, 128), np.float32)
    for d in range(127):
        mdz[d, d] = -1.0
        mdz[d, d + 1] = 1.0
    mdz[127, 126] = -1.0
    mdz[127, 127] = 1.0
    mdzt = mdz.T.copy()

    bd = np.zeros((128, 128), np.float32)
    for dp in range(128):
        for k in range(7):
            d = dp + k - 3
            if 0 <= d < 128:
                bd[dp, d] = GK[k]

    # curl DoubleRow weights: identical rows applied to (hi, lo)
    cur = np.zeros((128, 4, 2, 128), np.float32)
    cur[:, 0, 0] = cur[:, 0, 1] = eye
    cur[:, 1, 0] = cur[:, 1, 1] = -eye
    cur[:, 2, 0] = cur[:, 2, 1] = mdzt
    cur[:, 3, 0] = cur[:, 3, 1] = -mdzt

    # pass1 (D+H) tap-pair weights, fp8
    kbd2 = np.zeros((128, 4, 2, 128), np.float32)
    for p in range(4):
        for i in range(2):
            k = 2 * p + i
            if k < 7:
                kbd2[:, p, i] = (GK[k] * bd).T

    # d pass2 (W) tap-pair weights, fp8
    gk8 = GK.astype(E4).astype(np.float32)
    kd2 = np.zeros((128, 4, 2, 128), np.float32)
    for p in range(4):
        for i in range(2):
            k = 2 * p + i
            if k < 7:
                kd2[:, p, i] = gk8[k] * eye

    # vn pass2 taps, bf16
    kwb = np.zeros((128, 7, 128), np.float32)
    for k in range(7):
        kwb[:, k] = GK[k] * eye

    suf = (np.arange(128)[:, None] >= np.arange(128)[None, :]).astype(
        np.float32)
    mc = np.zeros((128, 128), np.float32)
    mc[0, 0], mc[0, 1] = -0.5, 0.5
    for k in range(1, 127):
        mc[k, k - 1], mc[k, k + 1] = -0.5, 0.5
    mc[127, 126], mc[127, 127] = -0.5, -0.5
    sufmct = np.stack([suf, mc.T], axis=1)  # [128, 2, 128]

    red = np.zeros((128, 2), np.float32)
    red[:, 0] = 1.0
    red[127, 1] = 1.0

    return {
        "cur": cur.astype(E4),
        "kbd2": kbd2.astype(E4),
        "kd2": kd2.astype(E4),
        "kwb": kwb.astype(BF),
        "sufmct": sufmct.astype(BF),
        "red": red.astype(BF),
    }


def _wap(t, off, dims):
    """Custom window AP on tile t: free-offset off, free dims [[stride,n]..]."""
    a = t[:]
    return AP(a.tensor, a.offset + off, [list(a.ap[0])] + [list(d) for d in dims])


# vhl free strides
VW, VH, VCH, VHL = 1, 129, 71 * 129, 3 * 71 * 129


def build_program():
    nc = bacc.Bacc("TRN2", target_bir_lowering=False, debug=False)

    vhl_in = nc.dram_tensor("vhl_in", [128, 2, 3, 71, 129], F8,
                            kind="ExternalInput")
    d8_in = nc.dram_tensor("d8_in", [128, 70, 128], F8, kind="ExternalInput")
    # packed constants: c8 = [cur | kbd2 | kd2]; cb = bf16 blob
    # cb layout (free elems): kwb 7*128 | sufmct 2*128 | m0 3*128 | m1 3*128
    # | red 2
    c8_in = nc.dram_tensor("c8_in", [128, 3, 4, 2, 128], F8,
                           kind="ExternalInput")
    cb_in = nc.dram_tensor("cb_in", [128, 1922], BF16, kind="ExternalInput")
    out_t = nc.dram_tensor("out", [1, 8192], F32, kind="ExternalOutput")

    with tile.TileContext(nc) as tc:
        with tc.tile_pool(name="persist", bufs=1) as pp:
            c8 = pp.tile([128, 3, 4, 2, 128], F8, tag="c8")
            cb = pp.tile([128, 1922], BF16, tag="cb")
            cur, kbd2, kd2 = c8[:, 0], c8[:, 1], c8[:, 2]
            kwb = _wap(cb, 0, [[128, 7], [1, 128]])
            sufmct = _wap(cb, 896, [[128, 2], [1, 128]])
            m0t = _wap(cb, 1152, [[128, 3], [1, 128]])
            m1t = _wap(cb, 1536, [[128, 3], [1, 128]])
            red = _wap(cb, 1920, [[1, 2]])
            d8 = pp.tile([128, 71, 128], F8, tag="d8")
            vhl = pp.tile([128, 2, 3, 71, 129], F8, tag="vhl")
            nc.sync.dma_start(c8[:, 0], c8_in[:, 0])
            nc.sync.dma_start(d8[:, 0:14, :], d8_in[:, 0:14, :])
            nc.sync.dma_start(vhl[:, :, :, 0:6, :], vhl_in[:, :, :, 0:6, :])
            nc.sync.dma_start(c8[:, 1:3], c8_in[:, 1:3])
            nc.sync.dma_start(vhl[:, :, :, 6:18, :], vhl_in[:, :, :, 6:18, :])
            nc.sync.dma_start(d8[:, 14:70, :], d8_in[:, 14:70, :])
            nc.sync.dma_start(cb[:], cb_in[:])
            for a, b in ((18, 30), (30, 42), (42, 54), (54, 66), (66, 71)):
                nc.sync.dma_start(vhl[:, :, :, a:b, :], vhl_in[:, :, :, a:b, :])

            vn8 = pp.tile([128, 71, 128], F8, tag="vn8")
            vn2 = pp.tile([128, 70, 128], BF16, tag="vn2")
            s1d8 = pp.tile([128, 65, 140], F8, tag="s1d8")
            s1b = pp.tile([128, 64, 134], BF16, tag="s1b")
            dsb = pp.tile([128, 64, 128], BF16, tag="dsb")
            vnsb = pp.tile([128, 64, 128], BF16, tag="vnsb")
            ivsb = pp.tile([1, 8192], F32, tag="ivsb")

            # zero the conv pads once (and slack rows read by zero-weight
            # DoubleRow rows)
            nc.gpsimd.memset(d8[:, 70:71, :], 0.0)
            nc.gpsimd.memset(vn8[:, 70:71, :], 0.0)
            nc.gpsimd.memset(s1d8[:, :, 0:3], 0.0)
            nc.gpsimd.memset(s1d8[:, :, 131:140], 0.0)
            nc.gpsimd.memset(s1d8[:, 64:65, :], 0.0)
            nc.gpsimd.memset(s1b[:, :, 0:3], 0.0)
            nc.gpsimd.memset(s1b[:, :, 131:134], 0.0)

            # ---- merged phase: d-smooth (both passes) interleaved with
            # curl. d-smooth is PE-heavy and Act-light; curl is the
            # opposite; interleaving keeps all engines fed. PSUM: shared
            # B/C pool (2 banks) + pcuv/pcw (6 banks) = 8.
            with tc.tile_pool(name="psBC", bufs=2,
                              space=bass.MemorySpace.PSUM) as psBCp, \
                 tc.tile_pool(name="psD", bufs=2,
                              space=bass.MemorySpace.PSUM) as psDp, \
                 tc.tile_pool(name="sqp", bufs=4) as sqp:
                KBD = [_wap(c8, p * 256, [[128, 2], [1, 128]])
                       for p in range(4)]
                KD = [_wap(c8, 2048 + p * 256, [[128, 2], [1, 128]])
                      for p in range(4)]
                IP2 = _wap(c8, 1024 + 0 * 256, [[128, 2], [1, 128]])
                IN2 = _wap(c8, 1024 + 1 * 256, [[128, 2], [1, 128]])
                MDZT2 = _wap(c8, 1024 + 2 * 256, [[128, 2], [1, 128]])
                MDZTN2 = _wap(c8, 1024 + 3 * 256, [[128, 2], [1, 128]])

                def vwin(ch, s, wo, cnt):
                    return _wap(vhl, ch * VCH + s * VH + wo,
                                [[VHL, 2], [VH, cnt], [1, 128]])

                def d_pass1(g4):
                    psB = psBCp.tile([128, 4, 128], F32, tag="psBC")
                    for p in range(4):
                        rhs = _wap(d8, (g4 + 2 * p) * 128,
                                   [[128, 2], [1, 512]])
                        nc.tensor.matmul(psB[:], KBD[p], rhs,
                                         start=(p == 0), stop=(p == 3),
                                         perf_mode=DR)
                    nc.scalar.copy(s1d8[:, g4:g4 + 4, 3:131], psB[:])

                def d_pass2(g4):
                    psC = psBCp.tile([128, 4, 128], F32, tag="psBC")
                    for p in range(4):
                        rhs = _wap(s1d8, g4 * 140 + 2 * p,
                                   [[1, 2], [140, 4], [1, 128]])
                        nc.tensor.matmul(psC[:], KD[p], rhs,
                                         start=(p == 0), stop=(p == 3),
                                         perf_mode=DR)
                    nc.vector.tensor_copy(dsb[:, g4:g4 + 4, :], psC[:])

                def curl_group(s0):
                    cnt = min(4, 70 - s0)
                    pcuv = psDp.tile([128, 2, 4, 128], F32, tag="pcuv")
                    pcw = psDp.tile([128, 4, 128], F32, tag="pcw")
                    ou, ov, ow = (pcuv[:, 0, 0:cnt], pcuv[:, 1, 0:cnt],
                                  pcw[:, 0:cnt])
                    nc.tensor.matmul(ou, IP2, vwin(2, s0 + 1, 0, cnt),
                                     start=True, stop=False, perf_mode=DR)
                    nc.tensor.matmul(ou, IN2, vwin(2, s0, 0, cnt),
                                     start=False, stop=False, perf_mode=DR)
                    nc.tensor.matmul(ou, MDZTN2, vwin(1, s0, 0, cnt),
                                     start=False, stop=True, perf_mode=DR)
                    nc.tensor.matmul(ov, MDZT2, vwin(0, s0, 0, cnt),
                                     start=True, stop=False, perf_mode=DR)
                    nc.tensor.matmul(ov, IN2, vwin(2, s0, 1, cnt),
                                     start=False, stop=False, perf_mode=DR)
                    nc.tensor.matmul(ov, IP2, vwin(2, s0, 0, cnt),
                                     start=False, stop=True, perf_mode=DR)

                    nc.tensor.matmul(ow, IP2, vwin(1, s0, 1, cnt),
                                     start=True, stop=False, perf_mode=DR)
                    nc.tensor.matmul(ow, IN2, vwin(1, s0, 0, cnt),
                                     start=False, stop=False, perf_mode=DR)
                    nc.tensor.matmul(ow, IN2, vwin(0, s0 + 1, 0, cnt),
                                     start=False, stop=False, perf_mode=DR)
                    nc.tensor.matmul(ow, IP2, vwin(0, s0, 0, cnt),
                                     start=False, stop=True, perf_mode=DR)

                    squv = sqp.tile([128, 2, 4, 128], BF16, tag="squv")
                    cwb = sqp.tile([128, 4, 128], BF16, tag="cwb")
                    sqw = sqp.tile([128, 4, 128], BF16, tag="sqw")
                    tsum = sqp.tile([128, 4, 128], BF16, tag="tsum")
                    au, av = squv[:, 0, 0:cnt], squv[:, 1, 0:cnt]
                    acw, aw = cwb[:, 0:cnt], sqw[:, 0:cnt]
                    ats = tsum[:, 0:cnt]
                    vn2g = vn2[:, s0:s0 + cnt, :]
                    nc.scalar.activation(squv[:, :, 0:cnt], pcuv[:, :, 0:cnt],
                                         AF.Square)
                    nc.vector.tensor_copy(acw, ow)
                    nc.vector.tensor_mul(aw, acw, acw)
                    nc.vector.tensor_add(ats, au, av)
                    nc.gpsimd.tensor_add(vn2g, ats, aw)
                    if s0 == 0:
                        nc.vector.tensor_mul(vn2[:, 0:3], vn2[:, 0:3], m0t)
                    if s0 <= 67 < s0 + cnt:
                        nc.vector.tensor_mul(
                            vn2[:, 67:s0 + cnt], vn2[:, 67:s0 + cnt],
                            _wap(cb, 1536, [[128, s0 + cnt - 67], [1, 128]]))
                    elif s0 > 67:
                        nc.vector.tensor_mul(
                            vn2[:, s0:s0 + cnt], vn2[:, s0:s0 + cnt],
                            _wap(cb, 1536 + (s0 - 67) * 128,
                                 [[128, cnt], [1, 128]]))
                    return s0 + cnt

                # note curl groups: s0 = 0,4,...,64 (17 groups of 4) + 68 (2)
                # front-load d-smooth so the PE queue never blocks behind the
                # first curl group's v DMA
                for g in range(2):
                    d_pass1(g * 4)
                d_pass2(0)
                def e_pre(cc):
                    # pre-run vn-smooth pass1 for the first E groups out of
                    # the (drained) B/C psum pool so their s1b copies queue
                    # on Act before the sqrt->exp table switch
                    psE = psBCp.tile([128, 4, 128], F32, tag="psBC")
                    g4 = cc * 4
                    for p in range(4):
                        rhs = _wap(vn8, (g4 + 2 * p) * 128,
                                   [[128, 2], [1, 512]])
                        nc.tensor.matmul(psE[:], KBD[p], rhs,
                                         start=(p == 0), stop=(p == 3),
                                         perf_mode=DR)
                    nc.scalar.copy(s1b[:, g4:g4 + 4, 3:131], psE[:])

                s0 = 0
                sqrt_chunks = [(0, 14), (14, 28), (28, 42), (42, 56),
                               (56, 66), (66, 70)]
                for it in range(18):
                    s0 = curl_group(s0)
                    if it + 2 <= 15:
                        d_pass1((it + 2) * 4)
                    if it + 1 <= 15:
                        d_pass2((it + 1) * 4)
                    while sqrt_chunks and sqrt_chunks[0][1] <= s0:
                        a, b = sqrt_chunks.pop(0)
                        nc.scalar.activation(vn8[:, a:b, :],
                                             vn2[:, a:b, :], AF.Sqrt)
                e_pre(0)
                e_pre(1)

            # ---- phase E: vn-smooth + transmittance integration ----
            # PE stream per iteration: p1(cc) SUF(cc) p2(cc) MCT(cc-1)
            # RED(cc-2): the exp/bcb/tb chain for group cc gets a full
            # iteration before MCT consumes tb, and pch gets another before
            # RED consumes it, so the PE never waits on elementwise.
            with tc.tile_pool(name="psA", bufs=2,
                              space=bass.MemorySpace.PSUM) as psAp, \
                 tc.tile_pool(name="psW", bufs=1,
                              space=bass.MemorySpace.PSUM) as psWp, \
                 tc.tile_pool(name="ps4", bufs=1,
                              space=bass.MemorySpace.PSUM) as ps4p, \
                 tc.tile_pool(name="psM", bufs=2,
                              space=bass.MemorySpace.PSUM) as psMp, \
                 tc.tile_pool(name="piv", bufs=2,
                              space=bass.MemorySpace.PSUM) as pivp, \
                 tc.tile_pool(name="st4", bufs=3) as st4:
                KW = [_wap(cb, k * 128, [[1, 128]]) for k in range(7)]
                SUFW = _wap(cb, 896, [[1, 128]])
                MCTW = _wap(cb, 1024, [[1, 128]])
                RED0 = _wap(cb, 1920, [[1, 1]])
                RED1 = _wap(cb, 1921, [[1, 1]])
                mct_q = []   # (g4, tb)
                red_q = []   # (g4, pch, cc)

                def do_mct(g4, tb):
                    psM = psMp.tile([128, 4, 128], F32, tag="psM")
                    nc.tensor.matmul(psM[:], MCTW, tb[:],
                                     start=True, stop=True)
                    pch = st4.tile([128, 4, 128], BF16, tag="pch")
                    nc.vector.tensor_mul(pch[:], psM[:],
                                         vnsb[:, g4:g4 + 4, :])
                    return pch

                def do_red(g4, pch, pcc):
                    pv = pivp.tile([1, 512], F32, tag="pv")
                    nc.tensor.matmul(pv[:], RED0, pch[:],
                                     start=True, stop=False)
                    nc.tensor.matmul(pv[:], RED1, vnsb[:, g4:g4 + 4, :],
                                     start=False, stop=True)
                    nc.vector.tensor_scalar(
                        ivsb[0:1, pcc * 512:(pcc + 1) * 512], pv[:],
                        1.0, 0.0, AL.min, AL.max)
                    if pcc == 7:
                        nc.sync.dma_start(out_t[0:1, 0:4096],
                                          ivsb[0:1, 0:4096])

                def do_p2(g4):
                    psW = psWp.tile([128, 4, 128], F32, tag="psW")
                    for k in range(7):
                        nc.tensor.matmul(psW[:], KW[k],
                                         s1b[:, g4:g4 + 4, k:k + 128],
                                         start=(k == 0), stop=(k == 6))
                    nc.scalar.copy(vnsb[:, g4:g4 + 4, :], psW[:])

                for cc in range(16):
                    g4 = cc * 4
                    if cc >= 2:
                        psA = psAp.tile([128, 4, 128], F32, tag="psA")
                        for p in range(4):
                            rhs = _wap(vn8, (g4 + 2 * p) * 128,
                                       [[128, 2], [1, 512]])
                            lhsT = _wap(c8, p * 256,
                                        [[128, 2], [1, 128]])
                            nc.tensor.matmul(psA[:], lhsT, rhs,
                                             start=(p == 0), stop=(p == 3),
                                             perf_mode=DR)
                    ps4 = ps4p.tile([128, 4, 128], F32, tag="ps4")
                    nc.tensor.matmul(ps4[:], SUFW, dsb[:, g4:g4 + 4, :],
                                     start=True, stop=True)
                    if cc >= 2:
                        nc.scalar.copy(s1b[:, g4:g4 + 4, 3:131], psA[:])
                    ecb = st4.tile([128, 4, 128], BF16, tag="ecb")
                    bcb = st4.tile([128, 4, 128], BF16, tag="bcb")
                    tb = st4.tile([128, 4, 128], BF16, tag="tb")
                    nc.scalar.activation(ecb[:], ps4[:], AF.Exp, scale=-C)
                    nc.vector.tensor_scalar(bcb[:], ps4[:], C, 1.0,
                                            AL.mult, AL.add)
                    nc.gpsimd.tensor_mul(tb[:], bcb[:], ecb[:])
                    if cc >= 1:
                        do_p2(g4 - 4)
                    if len(mct_q) > 1:
                        pg4, ptb = mct_q.pop(0)
                        pch = do_mct(pg4, ptb)
                        red_q.append((pg4, pch, pg4 // 4))
                    if len(red_q) > 1:
                        do_red(*red_q.pop(0))
                    mct_q.append((g4, tb))
                do_p2(60)
                while mct_q:
                    pg4, ptb = mct_q.pop(0)
                    pch = do_mct(pg4, ptb)
                    red_q.append((pg4, pch, pg4 // 4))
                while red_q:
                    do_red(*red_q.pop(0))
                nc.sync.dma_start(out_t[0:1, 4096:8192],
                                  ivsb[0:1, 4096:8192])

    nc.compile()
    return nc


def host_prepare(d_np, v_np):
    cm = _consts()
    c8 = np.stack([cm["kbd2"].astype(np.float32),
                   cm["cur"].astype(np.float32),
                   cm["kd2"].astype(np.float32)], axis=1).astype(E4)
    zeros3 = np.zeros((128, 3, 128), np.float32)
    ones3 = np.ones((128, 3, 128), np.float32)

    def cb_blob(hh):
        m0 = zeros3 if hh == 0 else ones3
        m1 = zeros3 if hh == 1 else ones3
        parts = [cm["kwb"].astype(np.float32).reshape(128, -1),
                 cm["sufmct"].astype(np.float32).reshape(128, -1),
                 m0.reshape(128, -1), m1.reshape(128, -1),
                 cm["red"].astype(np.float32).reshape(128, -1)]
        return np.concatenate(parts, axis=1).astype(BF)

    cb0, cb1 = cb_blob(0), cb_blob(1)
    cores = []
    vext = np.zeros((3, 128, 135, 129), np.float32)
    for c in range(8):
        b, hh = c // 2, c % 2
        h0 = 64 * hh
        dpad = np.zeros((128, 70, 128), np.float32)
        lo, hi = h0 - 3, h0 + 67
        src_lo, src_hi = max(lo, 0), min(hi, 128)
        dpad[:, (src_lo - lo):(src_hi - lo), :] = \
            d_np[b, 0, :, src_lo:src_hi, :]
        vext[:] = 0.0
        vext[:, :, 3:131, 0:128] = v_np[b]
        vext[:, :, 131, 0:128] = \
            2 * v_np[b, :, :, 127, :] - v_np[b, :, :, 126, :]
        vext[:, :, :, 128] = 2 * vext[:, :, :, 127] - vext[:, :, :, 126]
        vin = vext[:, :, h0:h0 + 71, :]  # [3, 128, 71, 129]
        vhi = vin.astype(E4)
        vlo = (vin - vhi.astype(np.float32)).astype(E4)
        vhl = np.stack([vhi, vlo], axis=0)  # [2, 3, 128, 71, 129]
        vhl = np.ascontiguousarray(vhl.transpose(2, 0, 1, 3, 4))
        m = {
            "vhl_in": vhl,
            "d8_in": dpad.astype(E4),
            "c8_in": c8,
            "cb_in": cb0 if hh == 0 else cb1,
        }
        cores.append(m)
    return cores


_NC = None


def kernel(d, v):
    global _NC
    d = np.asarray(d, np.float32)
    v = np.asarray(v, np.float32)
    if _NC is None:
        _NC = build_program()
    in_maps = host_prepare(d, v)
    res = run_bass_kernel_spmd(_NC, in_maps, list(range(8)))
    out = np.zeros((4, 1, 128, 128), np.float32)
    for c in range(8):
        b, hh = c // 2, c % 2
        out[b, 0, 64 * hh:64 * hh + 64, :] = \
            res.results[c]["out"].reshape(64, 128)
    return out


# revision 7
# speedup vs baseline: 1.0973x; 1.0196x over previous
"""Trainium2 Bass kernel for DiffVorticeSketchRender — fp8 DoubleRow version.

Sharding: 8 cores = 4 batches x 2 H-halves (64 rows each + 3-row halos).
Device layout: [D=128 partitions, H slices, W free] everywhere.

Key speedups vs the fp32r baseline:
- v shipped as fp8 hi/lo pairs (hi = e4m3(v), lo = e4m3(v - hi)); every curl
  matmul is a DoubleRow fp8 matmul (0.5 cycles/row) contracting hi and lo in
  one instruction with identical weight rows -> ~bf16 accuracy at 2x speed.
- gaussian smooths: pass1 (D-conv via band matrix fused with H-conv) runs as
  4 DoubleRow matmuls per group, two H-taps per instruction (fp8 weights).
  d-smooth pass2 (W-conv) also fp8 DoubleRow tap pairs; vn-smooth pass2 in
  bf16 (fp8 tap quantization there costs too much accuracy).
- elementwise work spread across Act/DVE/Pool in bf16; transmittance stage
  in bf16 matmuls.
"""

import numpy as np
import ml_dtypes

import concourse.bacc as bacc
import concourse.bass as bass
import concourse.mybir as mybir
import concourse.tile as tile
from bass_rust import AP
from concourse.bass_utils import run_bass_kernel_spmd

F32 = mybir.dt.float32
F8 = mybir.dt.float8e4
BF16 = mybir.dt.bfloat16
AL = mybir.AluOpType
AF = mybir.ActivationFunctionType
DR = mybir.MatmulPerfMode.DoubleRow

E4 = ml_dtypes.float8_e4m3
BF = ml_dtypes.bfloat16

KHS, SIGMA, C = 3, 1.6, 20.0


def _gauss1d():
    size = 2 * KHS + 1
    g = np.arange(size, dtype=np.float64) - (size - 1) / 2.0
    g = np.exp(-((g / SIGMA) ** 2) / 2.0) / (SIGMA * np.sqrt(2.0 * np.pi))
    return (g / g.sum()).astype(np.float32)


GK = _gauss1d()


def _consts():
    eye = np.eye(128, dtype=np.float32)
    mdz = np.zeros((128, 128), np.float32)
    for d in range(127):
        mdz[d, d] = -1.0
        mdz[d, d + 1] = 1.0
    mdz[127, 126] = -1.0
    mdz[127, 127] = 1.0
    mdzt = mdz.T.copy()

    bd = np.zeros((128, 128), np.float32)
    for dp in range(128):
        for k in range(7):
            d = dp + k - 3
            if 0 <= d < 128:
                bd[dp, d] = GK[k]

    # curl DoubleRow weights: identical rows applied to (hi, lo)
    cur = np.zeros((128, 4, 2, 128), np.float32)
    cur[:, 0, 0] = cur[:, 0, 1] = eye
    cur[:, 1, 0] = cur[:, 1, 1] = -eye
    cur[:, 2, 0] = cur[:, 2, 1] = mdzt
    cur[:, 3, 0] = cur[:, 3, 1] = -mdzt

    # pass1 (D+H) tap-pair weights, fp8
    kbd2 = np.zeros((128, 4, 2, 128), np.float32)
    for p in range(4):
        for i in range(2):
            k = 2 * p + i
            if k < 7:
                kbd2[:, p, i] = (GK[k] * bd).T

    # d pass2 (W) tap-pair weights, fp8
    gk8 = GK.astype(E4).astype(np.float32)
    kd2 = np.zeros((128, 4, 2, 128), np.float32)
    for p in range(4):
        for i in range(2):
            k = 2 * p + i
            if k < 7:
                kd2[:, p, i] = gk8[k] * eye

    # vn pass2 taps, bf16
    kwb = np.zeros((128, 7, 128), np.float32)
    for k in range(7):
        kwb[:, k] = GK[k] * eye

    suf = (np.arange(128)[:, None] >= np.arange(128)[None, :]).astype(
        np.float32)
    mc = np.zeros((128, 128), np.float32)
    mc[0, 0], mc[0, 1] = -0.5, 0.5
    for k in range(1, 127):
        mc[k, k - 1], mc[k, k + 1] = -0.5, 0.5
    mc[127, 126], mc[127, 127] = -0.5, -0.5
    sufmct = np.stack([suf, mc.T], axis=1)  # [128, 2, 128]

    red = np.zeros((128, 2), np.float32)
    red[:, 0] = 1.0
    red[127, 1] = 1.0

    return {
        "cur": cur.astype(E4),
        "kbd2": kbd2.astype(E4),
        "kd2": kd2.astype(E4),
        "kwb": kwb.astype(BF),
        "sufmct": sufmct.astype(BF),
        "red": red.astype(BF),
    }


def _wap(t, off, dims):
    """Custom window AP on tile t: free-offset off, free dims [[stride,n]..]."""
    a = t[:]
    return AP(a.tensor, a.offset + off, [list(a.ap[0])] + [list(d) for d in dims])


# vhl free strides
VW, VH, VCH, VHL = 1, 129, 71 * 129, 3 * 71 * 129


def build_program():
    nc = bacc.Bacc("TRN2", target_bir_lowering=False, debug=False)

    vhl_in = nc.dram_tensor("vhl_in", [128, 2, 3, 71, 129], F8,
                            kind="ExternalInput")
    d8_in = nc.dram_tensor("d8_in", [128, 70, 128], F8, kind="ExternalInput")
    # packed constants: c8 = [cur | kbd2 | kd2]; cb = bf16 blob
    # cb layout (free elems): kwb 7*128 | sufmct 2*128 | m0 3*128 | m1 3*128
    # | red 2
    c8_in = nc.dram_tensor("c8_in", [128, 3, 4, 2, 128], F8,
                           kind="ExternalInput")
    cb_in = nc.dram_tensor("cb_in", [128, 1922], BF16, kind="ExternalInput")
    out_t = nc.dram_tensor("out", [1, 8192], F32, kind="ExternalOutput")

    with tile.TileContext(nc) as tc:
        with tc.tile_pool(name="persist", bufs=1) as pp:
            c8 = pp.tile([128, 3, 4, 2, 128], F8, tag="c8")
            cb = pp.tile([128, 1922], BF16, tag="cb")
            cur, kbd2, kd2 = c8[:, 0], c8[:, 1], c8[:, 2]
            kwb = _wap(cb, 0, [[128, 7], [1, 128]])
            sufmct = _wap(cb, 896, [[128, 2], [1, 128]])
            m0t = _wap(cb, 1152, [[128, 3], [1, 128]])
            m1t = _wap(cb, 1536, [[128, 3], [1, 128]])
            red = _wap(cb, 1920, [[1, 2]])
            d8 = pp.tile([128, 71, 128], F8, tag="d8")
            vhl = pp.tile([128, 2, 3, 71, 129], F8, tag="vhl")
            nc.sync.dma_start(c8[:, 0], c8_in[:, 0])
            nc.sync.dma_start(d8[:, 0:22, :], d8_in[:, 0:22, :])
            nc.sync.dma_start(vhl[:, :, :, 0:6, :], vhl_in[:, :, :, 0:6, :])
            nc.sync.dma_start(c8[:, 1:3], c8_in[:, 1:3])
            nc.sync.dma_start(vhl[:, :, :, 6:18, :], vhl_in[:, :, :, 6:18, :])
            nc.sync.dma_start(cb[:, 1152:1920], cb_in[:, 1152:1920])
            nc.sync.dma_start(d8[:, 22:70, :], d8_in[:, 22:70, :])
            for a, b in ((18, 30), (30, 42), (42, 54), (54, 66), (66, 71)):
                nc.sync.dma_start(vhl[:, :, :, a:b, :], vhl_in[:, :, :, a:b, :])
            nc.sync.dma_start(cb[:, 0:1152], cb_in[:, 0:1152])
            nc.sync.dma_start(cb[:, 1920:1922], cb_in[:, 1920:1922])

            vn8 = pp.tile([128, 71, 128], F8, tag="vn8")
            vn2 = pp.tile([128, 70, 128], BF16, tag="vn2")
            s1d8 = pp.tile([128, 65, 140], F8, tag="s1d8")
            s1b = pp.tile([128, 64, 134], BF16, tag="s1b")
            dsb = pp.tile([128, 64, 128], BF16, tag="dsb")
            vnsb = pp.tile([128, 64, 128], BF16, tag="vnsb")
            ivsb = pp.tile([1, 8192], F32, tag="ivsb")

            # zero the conv pads once (and slack rows read by zero-weight
            # DoubleRow rows)
            nc.gpsimd.memset(d8[:, 70:71, :], 0.0)
            nc.gpsimd.memset(vn8[:, 70:71, :], 0.0)
            nc.gpsimd.memset(s1d8[:, :, 0:3], 0.0)
            nc.gpsimd.memset(s1d8[:, :, 131:140], 0.0)
            nc.gpsimd.memset(s1d8[:, 64:65, :], 0.0)
            nc.gpsimd.memset(s1b[:, :, 0:3], 0.0)
            nc.gpsimd.memset(s1b[:, :, 131:134], 0.0)

            # ---- merged phase: d-smooth (both passes) interleaved with
            # curl. d-smooth is PE-heavy and Act-light; curl is the
            # opposite; interleaving keeps all engines fed. PSUM: shared
            # B/C pool (2 banks) + pcuv/pcw (6 banks) = 8.
            with tc.tile_pool(name="psBC", bufs=2,
                              space=bass.MemorySpace.PSUM) as psBCp, \
                 tc.tile_pool(name="psD", bufs=2,
                              space=bass.MemorySpace.PSUM) as psDp, \
                 tc.tile_pool(name="sqp", bufs=4) as sqp:
                KBD = [_wap(c8, p * 256, [[128, 2], [1, 128]])
                       for p in range(4)]
                KD = [_wap(c8, 2048 + p * 256, [[128, 2], [1, 128]])
                      for p in range(4)]
                IP2 = _wap(c8, 1024 + 0 * 256, [[128, 2], [1, 128]])
                IN2 = _wap(c8, 1024 + 1 * 256, [[128, 2], [1, 128]])
                MDZT2 = _wap(c8, 1024 + 2 * 256, [[128, 2], [1, 128]])
                MDZTN2 = _wap(c8, 1024 + 3 * 256, [[128, 2], [1, 128]])

                def vwin(ch, s, wo, cnt):
                    return _wap(vhl, ch * VCH + s * VH + wo,
                                [[VHL, 2], [VH, cnt], [1, 128]])

                def d_pass1(g4):
                    psB = psBCp.tile([128, 4, 128], F32, tag="psBC")
                    for p in range(4):
                        rhs = _wap(d8, (g4 + 2 * p) * 128,
                                   [[128, 2], [1, 512]])
                        nc.tensor.matmul(psB[:], KBD[p], rhs,
                                         start=(p == 0), stop=(p == 3),
                                         perf_mode=DR)
                    nc.scalar.copy(s1d8[:, g4:g4 + 4, 3:131], psB[:])

                def d_pass2(g4):
                    psC = psBCp.tile([128, 4, 128], F32, tag="psBC")
                    for p in range(4):
                        rhs = _wap(s1d8, g4 * 140 + 2 * p,
                                   [[1, 2], [140, 4], [1, 128]])
                        nc.tensor.matmul(psC[:], KD[p], rhs,
                                         start=(p == 0), stop=(p == 3),
                                         perf_mode=DR)
                    nc.vector.tensor_copy(dsb[:, g4:g4 + 4, :], psC[:])

                def curl_group(s0):
                    cnt = min(4, 70 - s0)
                    pcuv = psDp.tile([128, 2, 4, 128], F32, tag="pcuv")
                    pcw = psDp.tile([128, 4, 128], F32, tag="pcw")
                    ou, ov, ow = (pcuv[:, 0, 0:cnt], pcuv[:, 1, 0:cnt],
                                  pcw[:, 0:cnt])
                    nc.tensor.matmul(ou, IP2, vwin(2, s0 + 1, 0, cnt),
                                     start=True, stop=False, perf_mode=DR)
                    nc.tensor.matmul(ou, IN2, vwin(2, s0, 0, cnt),
                                     start=False, stop=False, perf_mode=DR)
                    nc.tensor.matmul(ou, MDZTN2, vwin(1, s0, 0, cnt),
                                     start=False, stop=True, perf_mode=DR)
                    nc.tensor.matmul(ov, MDZT2, vwin(0, s0, 0, cnt),
                                     start=True, stop=False, perf_mode=DR)
                    nc.tensor.matmul(ov, IN2, vwin(2, s0, 1, cnt),
                                     start=False, stop=False, perf_mode=DR)
                    nc.tensor.matmul(ov, IP2, vwin(2, s0, 0, cnt),
                                     start=False, stop=True, perf_mode=DR)

                    nc.tensor.matmul(ow, IP2, vwin(1, s0, 1, cnt),
                                     start=True, stop=False, perf_mode=DR)
                    nc.tensor.matmul(ow, IN2, vwin(1, s0, 0, cnt),
                                     start=False, stop=False, perf_mode=DR)
                    nc.tensor.matmul(ow, IN2, vwin(0, s0 + 1, 0, cnt),
                                     start=False, stop=False, perf_mode=DR)
                    nc.tensor.matmul(ow, IP2, vwin(0, s0, 0, cnt),
                                     start=False, stop=True, perf_mode=DR)

                    squv = sqp.tile([128, 2, 4, 128], BF16, tag="squv")
                    cwb = sqp.tile([128, 4, 128], BF16, tag="cwb")
                    sqw = sqp.tile([128, 4, 128], BF16, tag="sqw")
                    tsum = sqp.tile([128, 4, 128], BF16, tag="tsum")
                    au, av = squv[:, 0, 0:cnt], squv[:, 1, 0:cnt]
                    acw, aw = cwb[:, 0:cnt], sqw[:, 0:cnt]
                    ats = tsum[:, 0:cnt]
                    vn2g = vn2[:, s0:s0 + cnt, :]
                    nc.scalar.activation(squv[:, :, 0:cnt], pcuv[:, :, 0:cnt],
                                         AF.Square)
                    nc.vector.tensor_copy(acw, ow)
                    nc.vector.tensor_mul(aw, acw, acw)
                    nc.vector.tensor_add(ats, au, av)
                    nc.gpsimd.tensor_add(vn2g, ats, aw)
                    if s0 == 0:
                        nc.vector.tensor_mul(vn2[:, 0:3], vn2[:, 0:3], m0t)
                    if s0 <= 67 < s0 + cnt:
                        nc.vector.tensor_mul(
                            vn2[:, 67:s0 + cnt], vn2[:, 67:s0 + cnt],
                            _wap(cb, 1536, [[128, s0 + cnt - 67], [1, 128]]))
                    elif s0 > 67:
                        nc.vector.tensor_mul(
                            vn2[:, s0:s0 + cnt], vn2[:, s0:s0 + cnt],
                            _wap(cb, 1536 + (s0 - 67) * 128,
                                 [[128, cnt], [1, 128]]))
                    return s0 + cnt

                # note curl groups: s0 = 0,4,...,64 (17 groups of 4) + 68 (2)
                # front-load d-smooth so the PE queue never blocks behind the
                # first curl group's v DMA
                for g in range(3):
                    d_pass1(g * 4)
                for g in range(2):
                    d_pass2(g * 4)
                def e_pre(cc):
                    # pre-run vn-smooth pass1 for the first E groups out of
                    # the (drained) B/C psum pool so their s1b copies queue
                    # on Act before the sqrt->exp table switch
                    psE = psBCp.tile([128, 4, 128], F32, tag="psBC")
                    g4 = cc * 4
                    for p in range(4):
                        rhs = _wap(vn8, (g4 + 2 * p) * 128,
                                   [[128, 2], [1, 512]])
                        nc.tensor.matmul(psE[:], KBD[p], rhs,
                                         start=(p == 0), stop=(p == 3),
                                         perf_mode=DR)
                    nc.scalar.copy(s1b[:, g4:g4 + 4, 3:131], psE[:])

                s0 = 0
                sqrt_chunks = [(0, 16), (16, 32), (32, 48), (48, 64),
                               (64, 67), (67, 70)]
                for it in range(18):
                    s0 = curl_group(s0)
                    if it + 3 <= 15:
                        d_pass1((it + 3) * 4)
                    if it + 2 <= 15:
                        d_pass2((it + 2) * 4)
                    while sqrt_chunks and sqrt_chunks[0][1] <= s0:
                        a, b = sqrt_chunks.pop(0)
                        nc.scalar.activation(vn8[:, a:b, :],
                                             vn2[:, a:b, :], AF.Sqrt)
                e_pre(0)
                e_pre(1)

            # ---- phase E: vn-smooth + transmittance integration ----
            # PE stream per iteration: p1(cc) SUF(cc) p2(cc) MCT(cc-1)
            # RED(cc-2): the exp/bcb/tb chain for group cc gets a full
            # iteration before MCT consumes tb, and pch gets another before
            # RED consumes it, so the PE never waits on elementwise.
            with tc.tile_pool(name="psA", bufs=2,
                              space=bass.MemorySpace.PSUM) as psAp, \
                 tc.tile_pool(name="psW", bufs=1,
                              space=bass.MemorySpace.PSUM) as psWp, \
                 tc.tile_pool(name="ps4", bufs=1,
                              space=bass.MemorySpace.PSUM) as ps4p, \
                 tc.tile_pool(name="psM", bufs=2,
                              space=bass.MemorySpace.PSUM) as psMp, \
                 tc.tile_pool(name="piv", bufs=2,
                              space=bass.MemorySpace.PSUM) as pivp, \
                 tc.tile_pool(name="st4", bufs=3) as st4:
                KW = [_wap(cb, k * 128, [[1, 128]]) for k in range(7)]
                SUFW = _wap(cb, 896, [[1, 128]])
                MCTW = _wap(cb, 1024, [[1, 128]])
                RED0 = _wap(cb, 1920, [[1, 1]])
                RED1 = _wap(cb, 1921, [[1, 1]])
                mct_q = []   # (g4, tb)
                red_q = []   # (g4, pch, cc)

                def do_mct(g4, tb):
                    psM = psMp.tile([128, 4, 128], F32, tag="psM")
                    nc.tensor.matmul(psM[:], MCTW, tb[:],
                                     start=True, stop=True)
                    pch = st4.tile([128, 4, 128], BF16, tag="pch")
                    nc.vector.tensor_mul(pch[:], psM[:],
                                         vnsb[:, g4:g4 + 4, :])
                    return pch

                def do_red(g4, pch, pcc):
                    pv = pivp.tile([1, 512], F32, tag="pv")
                    nc.tensor.matmul(pv[:], RED0, pch[:],
                                     start=True, stop=False)
                    nc.tensor.matmul(pv[:], RED1, vnsb[:, g4:g4 + 4, :],
                                     start=False, stop=True)
                    nc.vector.tensor_scalar(
                        ivsb[0:1, pcc * 512:(pcc + 1) * 512], pv[:],
                        1.0, 0.0, AL.min, AL.max)
                    if pcc == 7:
                        nc.sync.dma_start(out_t[0:1, 0:4096],
                                          ivsb[0:1, 0:4096])

                def do_p2(g4):
                    psW = psWp.tile([128, 4, 128], F32, tag="psW")
                    for k in range(7):
                        nc.tensor.matmul(psW[:], KW[k],
                                         s1b[:, g4:g4 + 4, k:k + 128],
                                         start=(k == 0), stop=(k == 6))
                    nc.scalar.copy(vnsb[:, g4:g4 + 4, :], psW[:])

                for cc in range(16):
                    g4 = cc * 4
                    if cc >= 2:
                        psA = psAp.tile([128, 4, 128], F32, tag="psA")
                        for p in range(4):
                            rhs = _wap(vn8, (g4 + 2 * p) * 128,
                                       [[128, 2], [1, 512]])
                            lhsT = _wap(c8, p * 256,
                                        [[128, 2], [1, 128]])
                            nc.tensor.matmul(psA[:], lhsT, rhs,
                                             start=(p == 0), stop=(p == 3),
                                             perf_mode=DR)
                    ps4 = ps4p.tile([128, 4, 128], F32, tag="ps4")
                    nc.tensor.matmul(ps4[:], SUFW, dsb[:, g4:g4 + 4, :],
                                     start=True, stop=True)
                    if cc >= 2:
                        nc.scalar.copy(s1b[:, g4:g4 + 4, 3:131], psA[:])
                    ecb = st4.tile([128, 4, 128], BF16, tag="ecb")
                    bcb = st4.tile([128, 4, 128], BF16, tag="bcb")
                    tb = st4.tile([128, 4, 128], BF16, tag="tb")
                    nc.scalar.activation(ecb[:], ps4[:], AF.Exp, scale=-C)
                    nc.vector.tensor_scalar(bcb[:], ps4[:], C, 1.0,
                                            AL.mult, AL.add)
                    nc.gpsimd.tensor_mul(tb[:], bcb[:], ecb[:])
                    if cc >= 1:
                        do_p2(g4 - 4)
                    if len(mct_q) > 1:
                        pg4, ptb = mct_q.pop(0)
                        pch = do_mct(pg4, ptb)
                        red_q.append((pg4, pch, pg4 // 4))
                    if len(red_q) > 1:
                        do_red(*red_q.pop(0))
                    mct_q.append((g4, tb))
                do_p2(60)
                while mct_q:
                    pg4, ptb = mct_q.pop(0)
                    pch = do_mct(pg4, ptb)
                    red_q.append((pg4, pch, pg4 // 4))
                while red_q:
                    do_red(*red_q.pop(0))
                nc.sync.dma_start(out_t[0:1, 4096:8192],
                                  ivsb[0:1, 4096:8192])

    nc.compile()
    return nc


def host_prepare(d_np, v_np):
    cm = _consts()
    c8 = np.stack([cm["kbd2"].astype(np.float32),
                   cm["cur"].astype(np.float32),
                   cm["kd2"].astype(np.float32)], axis=1).astype(E4)
    zeros3 = np.zeros((128, 3, 128), np.float32)
    ones3 = np.ones((128, 3, 128), np.float32)

    def cb_blob(hh):
        m0 = zeros3 if hh == 0 else ones3
        m1 = zeros3 if hh == 1 else ones3
        parts = [cm["kwb"].astype(np.float32).reshape(128, -1),
                 cm["sufmct"].astype(np.float32).reshape(128, -1),
                 m0.reshape(128, -1), m1.reshape(128, -1),
                 cm["red"].astype(np.float32).reshape(128, -1)]
        return np.concatenate(parts, axis=1).astype(BF)

    cb0, cb1 = cb_blob(0), cb_blob(1)
    cores = []
    vext = np.zeros((3, 128, 135, 129), np.float32)
    for c in range(8):
        b, hh = c // 2, c % 2
        h0 = 64 * hh
        dpad = np.zeros((128, 70, 128), np.float32)
        lo, hi = h0 - 3, h0 + 67
        src_lo, src_hi = max(lo, 0), min(hi, 128)
        dpad[:, (src_lo - lo):(src_hi - lo), :] = \
            d_np[b, 0, :, src_lo:src_hi, :]
        vext[:] = 0.0
        vext[:, :, 3:131, 0:128] = v_np[b]
        vext[:, :, 131, 0:128] = \
            2 * v_np[b, :, :, 127, :] - v_np[b, :, :, 126, :]
        vext[:, :, :, 128] = 2 * vext[:, :, :, 127] - vext[:, :, :, 126]
        vin = vext[:, :, h0:h0 + 71, :]  # [3, 128, 71, 129]
        vhi = vin.astype(E4)
        vlo = (vin - vhi.astype(np.float32)).astype(E4)
        vhl = np.stack([vhi, vlo], axis=0)  # [2, 3, 128, 71, 129]
        vhl = np.ascontiguousarray(vhl.transpose(2, 0, 1, 3, 4))
        m = {
            "vhl_in": vhl,
            "d8_in": dpad.astype(E4),
            "c8_in": c8,
            "cb_in": cb0 if hh == 0 else cb1,
        }
        cores.append(m)
    return cores


_NC = None


def kernel(d, v):
    global _NC
    d = np.asarray(d, np.float32)
    v = np.asarray(v, np.float32)
    if _NC is None:
        _NC = build_program()
    in_maps = host_prepare(d, v)
    res = run_bass_kernel_spmd(_NC, in_maps, list(range(8)))
    out = np.zeros((4, 1, 128, 128), np.float32)
    for c in range(8):
        b, hh = c // 2, c % 2
        out[b, 0, 64 * hh:64 * hh + 64, :] = \
            res.results[c]["out"].reshape(64, 128)
    return out


# revision 8
# speedup vs baseline: 1.1006x; 1.0029x over previous
"""Trainium2 Bass kernel for DiffVorticeSketchRender — fp8 DoubleRow version.

Sharding: 8 cores = 4 batches x 2 H-halves (64 rows each + 3-row halos).
Device layout: [D=128 partitions, H slices, W free] everywhere.

Key speedups vs the fp32r baseline:
- v shipped as fp8 hi/lo pairs (hi = e4m3(v), lo = e4m3(v - hi)); every curl
  matmul is a DoubleRow fp8 matmul (0.5 cycles/row) contracting hi and lo in
  one instruction with identical weight rows -> ~bf16 accuracy at 2x speed.
- gaussian smooths: pass1 (D-conv via band matrix fused with H-conv) runs as
  4 DoubleRow matmuls per group, two H-taps per instruction (fp8 weights).
  d-smooth pass2 (W-conv) also fp8 DoubleRow tap pairs; vn-smooth pass2 in
  bf16 (fp8 tap quantization there costs too much accuracy).
- elementwise work spread across Act/DVE/Pool in bf16; transmittance stage
  in bf16 matmuls.
"""

import numpy as np
import ml_dtypes

import concourse.bacc as bacc
import concourse.bass as bass
import concourse.mybir as mybir
import concourse.tile as tile
from bass_rust import AP
from concourse.bass_utils import run_bass_kernel_spmd

F32 = mybir.dt.float32
F8 = mybir.dt.float8e4
BF16 = mybir.dt.bfloat16
AL = mybir.AluOpType
AF = mybir.ActivationFunctionType
DR = mybir.MatmulPerfMode.DoubleRow

E4 = ml_dtypes.float8_e4m3
BF = ml_dtypes.bfloat16

KHS, SIGMA, C = 3, 1.6, 20.0


def _gauss1d():
    size = 2 * KHS + 1
    g = np.arange(size, dtype=np.float64) - (size - 1) / 2.0
    g = np.exp(-((g / SIGMA) ** 2) / 2.0) / (SIGMA * np.sqrt(2.0 * np.pi))
    return (g / g.sum()).astype(np.float32)


GK = _gauss1d()


def _consts():
    eye = np.eye(128, dtype=np.float32)
    mdz = np.zeros((128, 128), np.float32)
    for d in range(127):
        mdz[d, d] = -1.0
        mdz[d, d + 1] = 1.0
    mdz[127, 126] = -1.0
    mdz[127, 127] = 1.0
    mdzt = mdz.T.copy()

    bd = np.zeros((128, 128), np.float32)
    for dp in range(128):
        for k in range(7):
            d = dp + k - 3
            if 0 <= d < 128:
                bd[dp, d] = GK[k]

    # curl DoubleRow weights: identical rows applied to (hi, lo)
    cur = np.zeros((128, 4, 2, 128), np.float32)
    cur[:, 0, 0] = cur[:, 0, 1] = eye
    cur[:, 1, 0] = cur[:, 1, 1] = -eye
    cur[:, 2, 0] = cur[:, 2, 1] = mdzt
    cur[:, 3, 0] = cur[:, 3, 1] = -mdzt

    # pass1 (D+H) tap-pair weights, fp8
    kbd2 = np.zeros((128, 4, 2, 128), np.float32)
    for p in range(4):
        for i in range(2):
            k = 2 * p + i
            if k < 7:
                kbd2[:, p, i] = (GK[k] * bd).T

    # d pass2 (W) tap-pair weights, fp8
    gk8 = GK.astype(E4).astype(np.float32)
    kd2 = np.zeros((128, 4, 2, 128), np.float32)
    for p in range(4):
        for i in range(2):
            k = 2 * p + i
            if k < 7:
                kd2[:, p, i] = gk8[k] * eye

    # vn pass2 taps, bf16
    kwb = np.zeros((128, 7, 128), np.float32)
    for k in range(7):
        kwb[:, k] = GK[k] * eye

    suf = (np.arange(128)[:, None] >= np.arange(128)[None, :]).astype(
        np.float32)
    mc = np.zeros((128, 128), np.float32)
    mc[0, 0], mc[0, 1] = -0.5, 0.5
    for k in range(1, 127):
        mc[k, k - 1], mc[k, k + 1] = -0.5, 0.5
    mc[127, 126], mc[127, 127] = -0.5, -0.5
    sufmct = np.stack([suf, mc.T], axis=1)  # [128, 2, 128]

    red = np.zeros((128, 2), np.float32)
    red[:, 0] = 1.0
    red[127, 1] = 1.0

    return {
        "cur": cur.astype(E4),
        "kbd2": kbd2.astype(E4),
        "kd2": kd2.astype(E4),
        "kwb": kwb.astype(BF),
        "sufmct": sufmct.astype(BF),
        "red": red.astype(BF),
    }


def _wap(t, off, dims):
    """Custom window AP on tile t: free-offset off, free dims [[stride,n]..]."""
    a = t[:]
    return AP(a.tensor, a.offset + off, [list(a.ap[0])] + [list(d) for d in dims])


# vhl free strides
VW, VH, VCH, VHL = 1, 129, 71 * 129, 3 * 71 * 129


def build_program():
    nc = bacc.Bacc("TRN2", target_bir_lowering=False, debug=False)

    vhl_in = nc.dram_tensor("vhl_in", [128, 2, 3, 71, 129], F8,
                            kind="ExternalInput")
    d8_in = nc.dram_tensor("d8_in", [128, 70, 128], F8, kind="ExternalInput")
    # packed constants: c8 = [cur | kbd2 | kd2]; cb = bf16 blob
    # cb layout (free elems): kwb 7*128 | sufmct 2*128 | m0 3*128 | m1 3*128
    # | red 2
    c8_in = nc.dram_tensor("c8_in", [128, 3, 4, 2, 128], F8,
                           kind="ExternalInput")
    cb_in = nc.dram_tensor("cb_in", [128, 1922], BF16, kind="ExternalInput")
    out_t = nc.dram_tensor("out", [1, 8192], F32, kind="ExternalOutput")

    with tile.TileContext(nc) as tc:
        with tc.tile_pool(name="persist", bufs=1) as pp:
            c8 = pp.tile([128, 3, 4, 2, 128], F8, tag="c8")
            cb = pp.tile([128, 1922], BF16, tag="cb")
            cur, kbd2, kd2 = c8[:, 0], c8[:, 1], c8[:, 2]
            kwb = _wap(cb, 0, [[128, 7], [1, 128]])
            sufmct = _wap(cb, 896, [[128, 2], [1, 128]])
            m0t = _wap(cb, 1152, [[128, 3], [1, 128]])
            m1t = _wap(cb, 1536, [[128, 3], [1, 128]])
            red = _wap(cb, 1920, [[1, 2]])
            d8 = pp.tile([128, 71, 128], F8, tag="d8")
            vhl = pp.tile([128, 2, 3, 71, 129], F8, tag="vhl")
            nc.sync.dma_start(c8[:, 0], c8_in[:, 0])
            nc.sync.dma_start(d8[:, 0:22, :], d8_in[:, 0:22, :])
            nc.sync.dma_start(vhl[:, :, :, 0:6, :], vhl_in[:, :, :, 0:6, :])
            nc.sync.dma_start(c8[:, 1:3], c8_in[:, 1:3])
            nc.sync.dma_start(vhl[:, :, :, 6:18, :], vhl_in[:, :, :, 6:18, :])
            nc.sync.dma_start(cb[:, 1152:1920], cb_in[:, 1152:1920])
            nc.sync.dma_start(d8[:, 22:70, :], d8_in[:, 22:70, :])
            for a, b in ((18, 30), (30, 42), (42, 54), (54, 66), (66, 71)):
                nc.sync.dma_start(vhl[:, :, :, a:b, :], vhl_in[:, :, :, a:b, :])
            nc.sync.dma_start(cb[:, 0:1152], cb_in[:, 0:1152])
            nc.sync.dma_start(cb[:, 1920:1922], cb_in[:, 1920:1922])

            vn8 = pp.tile([128, 71, 128], F8, tag="vn8")
            vn2 = pp.tile([128, 70, 128], BF16, tag="vn2")
            s1d8 = pp.tile([128, 65, 140], F8, tag="s1d8")
            s1b = pp.tile([128, 64, 134], BF16, tag="s1b")
            dsb = pp.tile([128, 64, 128], BF16, tag="dsb")
            vnsb = pp.tile([128, 64, 128], BF16, tag="vnsb")
            ivsb = pp.tile([1, 8192], F32, tag="ivsb")

            # zero the conv pads once (and slack rows read by zero-weight
            # DoubleRow rows)
            nc.gpsimd.memset(d8[:, 70:71, :], 0.0)
            nc.gpsimd.memset(vn8[:, 70:71, :], 0.0)
            nc.gpsimd.memset(s1d8[:, :, 0:3], 0.0)
            nc.gpsimd.memset(s1d8[:, :, 131:140], 0.0)
            nc.gpsimd.memset(s1d8[:, 64:65, :], 0.0)
            nc.gpsimd.memset(s1b[:, :, 0:3], 0.0)
            nc.gpsimd.memset(s1b[:, :, 131:134], 0.0)

            # ---- merged phase: d-smooth (both passes) interleaved with
            # curl. d-smooth is PE-heavy and Act-light; curl is the
            # opposite; interleaving keeps all engines fed. PSUM: shared
            # B/C pool (2 banks) + pcuv/pcw (6 banks) = 8.
            with tc.tile_pool(name="psBC", bufs=2,
                              space=bass.MemorySpace.PSUM) as psBCp, \
                 tc.tile_pool(name="psD", bufs=2,
                              space=bass.MemorySpace.PSUM) as psDp, \
                 tc.tile_pool(name="sqp", bufs=4) as sqp:
                KBD = [_wap(c8, p * 256, [[128, 2], [1, 128]])
                       for p in range(4)]
                KD = [_wap(c8, 2048 + p * 256, [[128, 2], [1, 128]])
                      for p in range(4)]
                IP2 = _wap(c8, 1024 + 0 * 256, [[128, 2], [1, 128]])
                IN2 = _wap(c8, 1024 + 1 * 256, [[128, 2], [1, 128]])
                MDZT2 = _wap(c8, 1024 + 2 * 256, [[128, 2], [1, 128]])
                MDZTN2 = _wap(c8, 1024 + 3 * 256, [[128, 2], [1, 128]])

                def vwin(ch, s, wo, cnt):
                    return _wap(vhl, ch * VCH + s * VH + wo,
                                [[VHL, 2], [VH, cnt], [1, 128]])

                def d_pass1(g4):
                    psB = psBCp.tile([128, 4, 128], F32, tag="psBC")
                    for p in range(4):
                        rhs = _wap(d8, (g4 + 2 * p) * 128,
                                   [[128, 2], [1, 512]])
                        nc.tensor.matmul(psB[:], KBD[p], rhs,
                                         start=(p == 0), stop=(p == 3),
                                         perf_mode=DR)
                    nc.scalar.copy(s1d8[:, g4:g4 + 4, 3:131], psB[:])

                def d_pass2(g4):
                    psC = psBCp.tile([128, 4, 128], F32, tag="psBC")
                    for p in range(4):
                        rhs = _wap(s1d8, g4 * 140 + 2 * p,
                                   [[1, 2], [140, 4], [1, 128]])
                        nc.tensor.matmul(psC[:], KD[p], rhs,
                                         start=(p == 0), stop=(p == 3),
                                         perf_mode=DR)
                    nc.vector.tensor_copy(dsb[:, g4:g4 + 4, :], psC[:])

                def curl_group(s0):
                    cnt = min(4, 70 - s0)
                    pcuv = psDp.tile([128, 2, 4, 128], F32, tag="pcuv")
                    pcw = psDp.tile([128, 4, 128], F32, tag="pcw")
                    ou, ov, ow = (pcuv[:, 0, 0:cnt], pcuv[:, 1, 0:cnt],
                                  pcw[:, 0:cnt])
                    nc.tensor.matmul(ou, IP2, vwin(2, s0 + 1, 0, cnt),
                                     start=True, stop=False, perf_mode=DR)
                    nc.tensor.matmul(ou, IN2, vwin(2, s0, 0, cnt),
                                     start=False, stop=False, perf_mode=DR)
                    nc.tensor.matmul(ou, MDZTN2, vwin(1, s0, 0, cnt),
                                     start=False, stop=True, perf_mode=DR)
                    nc.tensor.matmul(ov, MDZT2, vwin(0, s0, 0, cnt),
                                     start=True, stop=False, perf_mode=DR)
                    nc.tensor.matmul(ov, IN2, vwin(2, s0, 1, cnt),
                                     start=False, stop=False, perf_mode=DR)
                    nc.tensor.matmul(ov, IP2, vwin(2, s0, 0, cnt),
                                     start=False, stop=True, perf_mode=DR)

                    nc.tensor.matmul(ow, IP2, vwin(1, s0, 1, cnt),
                                     start=True, stop=False, perf_mode=DR)
                    nc.tensor.matmul(ow, IN2, vwin(1, s0, 0, cnt),
                                     start=False, stop=False, perf_mode=DR)
                    nc.tensor.matmul(ow, IN2, vwin(0, s0 + 1, 0, cnt),
                                     start=False, stop=False, perf_mode=DR)
                    nc.tensor.matmul(ow, IP2, vwin(0, s0, 0, cnt),
                                     start=False, stop=True, perf_mode=DR)

                    squv = sqp.tile([128, 2, 4, 128], BF16, tag="squv")
                    cwb = sqp.tile([128, 4, 128], BF16, tag="cwb")
                    sqw = sqp.tile([128, 4, 128], BF16, tag="sqw")
                    tsum = sqp.tile([128, 4, 128], BF16, tag="tsum")
                    au, av = squv[:, 0, 0:cnt], squv[:, 1, 0:cnt]
                    acw, aw = cwb[:, 0:cnt], sqw[:, 0:cnt]
                    ats = tsum[:, 0:cnt]
                    vn2g = vn2[:, s0:s0 + cnt, :]
                    nc.scalar.activation(squv[:, :, 0:cnt], pcuv[:, :, 0:cnt],
                                         AF.Square)
                    nc.vector.tensor_copy(acw, ow)
                    nc.vector.tensor_mul(aw, acw, acw)
                    nc.vector.tensor_add(ats, au, av)
                    nc.gpsimd.tensor_add(vn2g, ats, aw)
                    if s0 == 0:
                        nc.vector.tensor_mul(vn2[:, 0:3], vn2[:, 0:3], m0t)
                    if s0 <= 67 < s0 + cnt:
                        nc.vector.tensor_mul(
                            vn2[:, 67:s0 + cnt], vn2[:, 67:s0 + cnt],
                            _wap(cb, 1536, [[128, s0 + cnt - 67], [1, 128]]))
                    elif s0 > 67:
                        nc.vector.tensor_mul(
                            vn2[:, s0:s0 + cnt], vn2[:, s0:s0 + cnt],
                            _wap(cb, 1536 + (s0 - 67) * 128,
                                 [[128, cnt], [1, 128]]))
                    return s0 + cnt

                # note curl groups: s0 = 0,4,...,64 (17 groups of 4) + 68 (2)
                # front-load d-smooth so the PE queue never blocks behind the
                # first curl group's v DMA
                for g in range(3):
                    d_pass1(g * 4)
                for g in range(2):
                    d_pass2(g * 4)
                def e_pre(cc):
                    # pre-run vn-smooth pass1 for the first E groups out of
                    # the (drained) B/C psum pool so their s1b copies queue
                    # on Act before the sqrt->exp table switch
                    psE = psBCp.tile([128, 4, 128], F32, tag="psBC")
                    g4 = cc * 4
                    for p in range(4):
                        rhs = _wap(vn8, (g4 + 2 * p) * 128,
                                   [[128, 2], [1, 512]])
                        nc.tensor.matmul(psE[:], KBD[p], rhs,
                                         start=(p == 0), stop=(p == 3),
                                         perf_mode=DR)
                    nc.scalar.copy(s1b[:, g4:g4 + 4, 3:131], psE[:])

                s0 = 0
                sqrt_chunks = [(0, 16), (16, 32), (32, 48), (48, 60),
                               (60, 67), (67, 70)]
                for it in range(18):
                    s0 = curl_group(s0)
                    if it + 3 <= 15:
                        d_pass1((it + 3) * 4)
                    if it + 2 <= 15:
                        d_pass2((it + 2) * 4)
                    while sqrt_chunks and sqrt_chunks[0][1] <= s0:
                        a, b = sqrt_chunks.pop(0)
                        nc.scalar.activation(vn8[:, a:b, :],
                                             vn2[:, a:b, :], AF.Sqrt)
                e_pre(0)
                e_pre(1)

            # ---- phase E: vn-smooth + transmittance integration ----
            # PE stream per iteration: p1(cc) SUF(cc) p2(cc) MCT(cc-1)
            # RED(cc-2): the exp/bcb/tb chain for group cc gets a full
            # iteration before MCT consumes tb, and pch gets another before
            # RED consumes it, so the PE never waits on elementwise.
            with tc.tile_pool(name="psA", bufs=2,
                              space=bass.MemorySpace.PSUM) as psAp, \
                 tc.tile_pool(name="psW", bufs=1,
                              space=bass.MemorySpace.PSUM) as psWp, \
                 tc.tile_pool(name="ps4", bufs=1,
                              space=bass.MemorySpace.PSUM) as ps4p, \
                 tc.tile_pool(name="psM", bufs=2,
                              space=bass.MemorySpace.PSUM) as psMp, \
                 tc.tile_pool(name="piv", bufs=2,
                              space=bass.MemorySpace.PSUM) as pivp, \
                 tc.tile_pool(name="st4", bufs=3) as st4:
                KW = [_wap(cb, k * 128, [[1, 128]]) for k in range(7)]
                SUFW = _wap(cb, 896, [[1, 128]])
                MCTW = _wap(cb, 1024, [[1, 128]])
                RED0 = _wap(cb, 1920, [[1, 1]])
                RED1 = _wap(cb, 1921, [[1, 1]])
                mct_q = []   # (g4, tb)
                red_q = []   # (g4, pch, cc)

                def do_mct(g4, tb):
                    psM = psMp.tile([128, 4, 128], F32, tag="psM")
                    nc.tensor.matmul(psM[:], MCTW, tb[:],
                                     start=True, stop=True)
                    pch = st4.tile([128, 4, 128], BF16, tag="pch")
                    nc.vector.tensor_mul(pch[:], psM[:],
                                         vnsb[:, g4:g4 + 4, :])
                    return pch

                def do_red(g4, pch, pcc):
                    pv = pivp.tile([1, 512], F32, tag="pv")
                    nc.tensor.matmul(pv[:], RED0, pch[:],
                                     start=True, stop=False)
                    nc.tensor.matmul(pv[:], RED1, vnsb[:, g4:g4 + 4, :],
                                     start=False, stop=True)
                    nc.vector.tensor_scalar(
                        ivsb[0:1, pcc * 512:(pcc + 1) * 512], pv[:],
                        1.0, 0.0, AL.min, AL.max)
                    if pcc == 7:
                        nc.sync.dma_start(out_t[0:1, 0:4096],
                                          ivsb[0:1, 0:4096])

                def do_p2(g4):
                    psW = psWp.tile([128, 4, 128], F32, tag="psW")
                    for k in range(7):
                        nc.tensor.matmul(psW[:], KW[k],
                                         s1b[:, g4:g4 + 4, k:k + 128],
                                         start=(k == 0), stop=(k == 6))
                    nc.scalar.copy(vnsb[:, g4:g4 + 4, :], psW[:])

                for cc in range(16):
                    g4 = cc * 4
                    if cc >= 2:
                        psA = psAp.tile([128, 4, 128], F32, tag="psA")
                        for p in range(4):
                            rhs = _wap(vn8, (g4 + 2 * p) * 128,
                                       [[128, 2], [1, 512]])
                            lhsT = _wap(c8, p * 256,
                                        [[128, 2], [1, 128]])
                            nc.tensor.matmul(psA[:], lhsT, rhs,
                                             start=(p == 0), stop=(p == 3),
                                             perf_mode=DR)
                    ps4 = ps4p.tile([128, 4, 128], F32, tag="ps4")
                    nc.tensor.matmul(ps4[:], SUFW, dsb[:, g4:g4 + 4, :],
                                     start=True, stop=True)
                    if cc >= 2:
                        nc.scalar.copy(s1b[:, g4:g4 + 4, 3:131], psA[:])
                    ecb = st4.tile([128, 4, 128], BF16, tag="ecb")
                    bcb = st4.tile([128, 4, 128], BF16, tag="bcb")
                    tb = st4.tile([128, 4, 128], BF16, tag="tb")
                    nc.scalar.activation(ecb[:], ps4[:], AF.Exp, scale=-C)
                    nc.vector.tensor_scalar(bcb[:], ps4[:], C, 1.0,
                                            AL.mult, AL.add)
                    nc.gpsimd.tensor_mul(tb[:], bcb[:], ecb[:])
                    if cc >= 1:
                        do_p2(g4 - 4)
                    if len(mct_q) > 1:
                        pg4, ptb = mct_q.pop(0)
                        pch = do_mct(pg4, ptb)
                        red_q.append((pg4, pch, pg4 // 4))
                    if len(red_q) > 1:
                        do_red(*red_q.pop(0))
                    mct_q.append((g4, tb))
                do_p2(60)
                while mct_q:
                    pg4, ptb = mct_q.pop(0)
                    pch = do_mct(pg4, ptb)
                    red_q.append((pg4, pch, pg4 // 4))
                while red_q:
                    do_red(*red_q.pop(0))
                nc.sync.dma_start(out_t[0:1, 4096:8192],
                                  ivsb[0:1, 4096:8192])

    nc.compile()
    return nc


def host_prepare(d_np, v_np):
    cm = _consts()
    c8 = np.stack([cm["kbd2"].astype(np.float32),
                   cm["cur"].astype(np.float32),
                   cm["kd2"].astype(np.float32)], axis=1).astype(E4)
    zeros3 = np.zeros((128, 3, 128), np.float32)
    ones3 = np.ones((128, 3, 128), np.float32)

    def cb_blob(hh):
        m0 = zeros3 if hh == 0 else ones3
        m1 = zeros3 if hh == 1 else ones3
        parts = [cm["kwb"].astype(np.float32).reshape(128, -1),
                 cm["sufmct"].astype(np.float32).reshape(128, -1),
                 m0.reshape(128, -1), m1.reshape(128, -1),
                 cm["red"].astype(np.float32).reshape(128, -1)]
        return np.concatenate(parts, axis=1).astype(BF)

    cb0, cb1 = cb_blob(0), cb_blob(1)
    cores = []
    vext = np.zeros((3, 128, 135, 129), np.float32)
    for c in range(8):
        b, hh = c // 2, c % 2
        h0 = 64 * hh
        dpad = np.zeros((128, 70, 128), np.float32)
        lo, hi = h0 - 3, h0 + 67
        src_lo, src_hi = max(lo, 0), min(hi, 128)
        dpad[:, (src_lo - lo):(src_hi - lo), :] = \
            d_np[b, 0, :, src_lo:src_hi, :]
        vext[:] = 0.0
        vext[:, :, 3:131, 0:128] = v_np[b]
        vext[:, :, 131, 0:128] = \
            2 * v_np[b, :, :, 127, :] - v_np[b, :, :, 126, :]
        vext[:, :, :, 128] = 2 * vext[:, :, :, 127] - vext[:, :, :, 126]
        vin = vext[:, :, h0:h0 + 71, :]  # [3, 128, 71, 129]
        vhi = vin.astype(E4)
        vlo = (vin - vhi.astype(np.float32)).astype(E4)
        vhl = np.stack([vhi, vlo], axis=0)  # [2, 3, 128, 71, 129]
        vhl = np.ascontiguousarray(vhl.transpose(2, 0, 1, 3, 4))
        m = {
            "vhl_in": vhl,
            "d8_in": dpad.astype(E4),
            "c8_in": c8,
            "cb_in": cb0 if hh == 0 else cb1,
        }
        cores.append(m)
    return cores


_NC = None


def kernel(d, v):
    global _NC
    d = np.asarray(d, np.float32)
    v = np.asarray(v, np.float32)
    if _NC is None:
        _NC = build_program()
    in_maps = host_prepare(d, v)
    res = run_bass_kernel_spmd(_NC, in_maps, list(range(8)))
    out = np.zeros((4, 1, 128, 128), np.float32)
    for c in range(8):
        b, hh = c // 2, c % 2
        out[b, 0, 64 * hh:64 * hh + 64, :] = \
            res.results[c]["out"].reshape(64, 128)
    return out
